# revision 19
# baseline (speedup 1.0000x reference)
"""DNC single-step forward on 8 Trainium2 NeuronCores (Bass/Tile).

Data-parallel over batch (B=256 -> 32/core); memory/link/usage/params
replicated. Cross-core collectives:
  - one AllGather: [allocation-weight shard | link row-sum shard]
  - one AllReduce: [erase/add matrix partials | link col-sum partial |
    masked-lu sum partial]

Math restructuring (validated vs reference at ~1e-6 in numpy):
  - h0=c0=0  =>  W_hh and the forget gate are dead.
  - prev_rw uniform 1/N  =>  read_flat = tiled col-mean(memory); the
    (B,R,N)@(N,N) backward/forward einsums collapse to (1/N)*col/row-sums
    of link_new.
  - lu = ww^T ww / B is symmetric; its diag-masked row/col sums reduce to
    (w^T s - sum_b w^2)/B with s = row-sums of w; N x N lu never formed.
  - allocation weights sort-free: alloc[i] = u[i]*exp(sum_{u_k<u_i} ln(1-u_k)).
"""
import sys

sys.path.insert(0, '/opt/trn_rl_repo')

import numpy as np
import concourse.bass as bass
import concourse.bacc as bacc
import concourse.tile as tile
from concourse import mybir
from concourse.bass_utils import run_bass_kernel_spmd
from concourse.masks import make_identity

AF = mybir.ActivationFunctionType
ALU = mybir.AluOpType
F32 = mybir.dt.float32

B, N, D, R, H, I = 256, 2048, 128, 4, 1024, 1024
CI = I + R * D          # 1536
IF = 787
M = 8                   # cores
BL = B // M             # 32 batch rows per core
NT = N // 128           # 16 n-tiles
KC = CI // 128          # 12 k-tiles of cin
KH = H // 128           # 8 k-tiles of h

ARA = NT * 128 * 256    # AR region A: [ep|ap] per n-tile
ARTOT = ARA + 2 * N     # + region B: [link colsum | lu stat]
DEBUG = False

# (1,N) vectors live at quarter partition offsets of two (128,N) tiles
# r0: usg@0, L@32, alloc@64, rowsum@96 ; r1: csum@0, lus@32, BW@64, FW@96


def build_nc():
    nc = bacc.Bacc("TRN2", target_bir_lowering=False, num_devices=M)
    dt = F32
    # ---- inputs (host-prepared layouts; per-partition contiguous DMA) ----
    cinx = nc.declare_dram_parameter("cinx", [128, KH * BL], dt, isOutput=False)
    w3 = nc.declare_dram_parameter("w3", [128, KC * 3072], dt, isOutput=False)
    b3c = nc.declare_dram_parameter("b3c", [128, 24], dt, isOutput=False)
    wif = nc.declare_dram_parameter("wif", [128, KH * IF], dt, isOutput=False)
    bifr = nc.declare_dram_parameter("bifr", [1, IF], dt, isOutput=False)
    wout = nc.declare_dram_parameter("wout", [128, KC * 1024], dt, isOutput=False)
    boc = nc.declare_dram_parameter("boc", [128, 8], dt, isOutput=False)
    memA = nc.declare_dram_parameter("memA", [128, N], dt, isOutput=False)
    lnk = nc.declare_dram_parameter("lnk", [128, 2 * N], dt, isOutput=False)
    usg = nc.declare_dram_parameter("usg", [1, N], dt, isOutput=False)
    ucols = nc.declare_dram_parameter("ucols", [128, 2], dt, isOutput=False)

    outT = nc.declare_dram_parameter("outT", [8, 128, BL], dt, isOutput=True)
    if DEBUG:
        d_hT = nc.declare_dram_parameter("d_hT", [128, KH * BL], dt, isOutput=True)
        d_itf = nc.declare_dram_parameter("d_itf", [32, IF], dt, isOutput=True)
        d_r0 = nc.declare_dram_parameter("d_r0", [128, N], dt, isOutput=True)
        d_mnew = nc.declare_dram_parameter("d_mnew", [128, N], dt, isOutput=True)
        d_wt = nc.declare_dram_parameter("d_wt", [32, N], dt, isOutput=True)
        d_cols = nc.declare_dram_parameter("d_cols", [128, 4], dt, isOutput=True)
        d_bw = nc.declare_dram_parameter("d_bw", [1, N], dt, isOutput=True)
        d_fw = nc.declare_dram_parameter("d_fw", [1, N], dt, isOutput=True)
        d_nrw = nc.declare_dram_parameter("d_nrw", [128, N], dt, isOutput=True)
        d_roT = nc.declare_dram_parameter("d_roT", [128, 128], dt, isOutput=True)
        d_cin = nc.declare_dram_parameter("d_cin", [128, KH * BL], dt, isOutput=True)
        d_cmean = nc.declare_dram_parameter("d_cmean", [128, BL], dt, isOutput=True)


    from contextlib import ExitStack
    with tile.TileContext(nc) as tc, ExitStack() as es:
        cons = es.enter_context(tc.tile_pool(name="cons", bufs=1))
        wk = es.enter_context(tc.tile_pool(name="wk", bufs=1))
        wstr = es.enter_context(tc.tile_pool(name="wstr", bufs=2))
        lstr = es.enter_context(tc.tile_pool(name="lstr", bufs=1))
        dpool = es.enter_context(tc.tile_pool(name="dram", bufs=1, space="DRAM"))
        pp1 = es.enter_context(tc.tile_pool(name="pp1", bufs=1, space="PSUM"))
        ppb = es.enter_context(tc.tile_pool(name="ppb", bufs=1, space="PSUM"))
        ppt = es.enter_context(tc.tile_pool(name="ppt", bufs=2, space="PSUM"))

        ag_in = dpool.tile([512], dt)
        ag_out = dpool.tile([M, 512], dt, addr_space="Shared")
        ar_in = dpool.tile([ARTOT], dt)
        ar_out = dpool.tile([ARTOT], dt, addr_space="Shared")
        arA_in = ar_in[0:ARA].rearrange("(t p f) -> t p f", p=128, f=256)
        arA_out = ar_out[0:ARA].rearrange("(t p f) -> t p f", p=128, f=256)

        ident = cons.tile([128, 128], dt)
        make_identity(nc, ident)
        ones_col = cons.tile([128, 1], dt)
        nc.vector.memset(ones_col, 1.0)
        ones32 = cons.tile([128, BL], dt)
        nc.vector.memset(ones32, 1.0)

        r0 = cons.tile([128, N], dt)
        r1 = cons.tile([128, N], dt)
        nc.sync.dma_start(out=r0[0:1, :], in_=usg[:, :])
        bif_sb = cons.tile([1, IF], dt)
        nc.sync.dma_start(out=bif_sb, in_=bifr[:, :])
        b3_sb = cons.tile([128, 24], dt)
        nc.sync.dma_start(out=b3_sb, in_=b3c[:, :])
        boc_sb = cons.tile([128, 8], dt)
        nc.sync.dma_start(out=boc_sb, in_=boc[:, :])
        mem_sb = cons.tile([128, N], dt)
        nc.sync.dma_start(out=mem_sb, in_=memA[:, :])
        uc_sb = cons.tile([128, 2], dt)
        nc.sync.dma_start(out=uc_sb, in_=ucols[:, :])

        usg_row = r0[0:1, :]
        tmp_row = r0[0:1, :]  # reused in stage E (usage dead by then)

        # ============ Stage A: alloc shard + link stats -> AllGather ========
        usg_b = wk.tile([128, N], dt, tag="usgb")
        nc.sync.dma_start(out=usg_b, in_=usg[0:1, :].partition_broadcast(128))
        L_b = wk.tile([128, N], dt, tag="lb")
        nc.scalar.activation(out=L_b, in_=usg_b, func=AF.Ln, bias=1.0,
                             scale=-1.0)
        for i in range(2):
            u_col = uc_sb[:, i:i + 1]
            step = wk.tile([128, N], dt, tag="step", bufs=1)
            nc.vector.tensor_scalar(out=step, in0=usg_b,
                                    scalar1=u_col, scalar2=None, op0=ALU.is_lt)
            nc.vector.tensor_tensor(out=step, in0=step, in1=L_b, op=ALU.mult)
            a_col = wk.tile([128, 1], dt, tag="acol", bufs=2)
            nc.vector.tensor_reduce(out=a_col, in_=step,
                                    axis=mybir.AxisListType.X, op=ALU.add)
            nc.scalar.activation(out=a_col, in_=a_col, func=AF.Exp)
            nc.vector.tensor_mul(out=a_col, in0=a_col, in1=u_col)
            nc.sync.dma_start(out=ag_in[i * 128:(i + 1) * 128], in_=a_col)

        cs_ps = ppb.tile([1, N], dt, tag="big")
        for i in range(2):
            lt = lstr.tile([128, N], dt, tag="lnk")
            nc.sync.dma_start(out=lt, in_=lnk[:, i * N:(i + 1) * N])
            rs = wk.tile([128, 1], dt, tag="rs", bufs=2)
            nc.vector.tensor_reduce(out=rs, in_=lt, axis=mybir.AxisListType.X,
                                    op=ALU.add)
            nc.sync.dma_start(out=ag_in[256 + i * 128:256 + (i + 1) * 128],
                              in_=rs)
            for ch in range(4):
                nc.tensor.matmul(cs_ps[0:1, ch * 512:(ch + 1) * 512],
                                 lhsT=ones_col,
                                 rhs=lt[:, ch * 512:(ch + 1) * 512],
                                 start=(i == 0), stop=(i == 1))
        cs_row = wk.tile([1, N], dt, tag="csrow")
        nc.scalar.copy(out=cs_row, in_=cs_ps)
        nc.sync.dma_start(out=ar_in[ARA:ARA + N], in_=cs_row)

        nc.gpsimd.collective_compute(
            "AllGather", ALU.bypass, replica_groups=[list(range(M))],
            ins=[ag_in[:]], outs=[ag_out.flatten()])
        nc.sync.dma_start(out=r0[64:65, :], in_=ag_out[:, 0:256])
        nc.sync.dma_start(out=r0[96:97, :], in_=ag_out[:, 256:512])
        alloc_row = r0[64:65, :]
        rowsum_row = r0[96:97, :]

        # ============ Stage B: LSTM ============
        cin = wk.tile([128, KH, BL], dt)
        nc.sync.dma_start(out=cin,
                          in_=cinx[:, :].rearrange("p (k b) -> p k b", b=BL))
        mean_ps = ppt.tile([1, 128], dt, tag="tr")
        for t in range(NT):
            nc.tensor.matmul(mean_ps, lhsT=ones_col,
                             rhs=mem_sb[:, t * 128:(t + 1) * 128],
                             start=(t == 0), stop=(t == NT - 1))
        mean_row = wk.tile([1, 128], dt, tag="meanr")
        nc.scalar.activation(out=mean_row, in_=mean_ps, func=AF.Copy,
                             scale=1.0 / N)
        mc_ps = ppt.tile([128, 1], dt, tag="tr")
        nc.tensor.transpose(mc_ps, mean_row, ident[0:1, 0:1])
        mean_col = wk.tile([128, 1], dt, tag="meanc")
        nc.vector.tensor_copy(out=mean_col, in_=mc_ps)
        cmean = wk.tile([128, BL], dt)
        nc.scalar.activation(out=cmean, in_=ones32, func=AF.Copy,
                             scale=mean_col)

        # j-outer / k-inner: one psum bank per accumulation group (a
        # start=True matmul claims a whole 2KB zero-region, so slices of one
        # bank cannot host interleaved groups).
        gsb = wk.tile([128, 24, BL], dt)   # activated gates, j-major
        for j in range(24):
            w3j = wstr.tile([128, KC * 128], dt, tag="w3j")
            nc.sync.dma_start(out=w3j,
                              in_=w3[:, j * KC * 128:(j + 1) * KC * 128])
            ps_j = ppt.tile([128, BL], dt, tag="tr")
            for k in range(KC):
                rhs_k = cin[:, k, :] if k < KH else cmean
                nc.tensor.matmul(ps_j, lhsT=w3j[:, k * 128:(k + 1) * 128],
                                 rhs=rhs_k,
                                 start=(k == 0), stop=(k == KC - 1))
            fn = AF.Tanh if 8 <= j < 16 else AF.Sigmoid
            nc.scalar.activation(out=gsb[:, j, :], in_=ps_j, func=fn,
                                 bias=b3_sb[:, j:j + 1])
        hT = wk.tile([128, KH, BL], dt)
        for t in range(KH):
            cc = wk.tile([128, BL], dt, tag="g1", bufs=2)
            nc.vector.tensor_mul(out=cc, in0=gsb[:, t, :], in1=gsb[:, 8 + t, :])
            nc.scalar.activation(out=cc, in_=cc, func=AF.Tanh)
            nc.vector.tensor_mul(out=hT[:, t, :], in0=cc, in1=gsb[:, 16 + t, :])

        # ============ Stage C: interface vector ============
        ps_itf = ppb.tile([32, IF], dt, tag="big")
        for k in range(KH):
            wfk = wstr.tile([128, IF], dt, tag="wifk")
            nc.sync.dma_start(out=wfk, in_=wif[:, k * IF:(k + 1) * IF])
            nc.tensor.matmul(ps_itf[:, 0:512], lhsT=hT[:, k, :],
                             rhs=wfk[:, 0:512], start=(k == 0),
                             stop=(k == KH - 1))
            nc.tensor.matmul(ps_itf[:, 512:IF], lhsT=hT[:, k, :],
                             rhs=wfk[:, 512:IF], start=(k == 0),
                             stop=(k == KH - 1))
        bif_b = wk.tile([32, IF], dt)
        nc.sync.dma_start(out=bif_b, in_=bifr[0:1, :].partition_broadcast(32))
        itf = wk.tile([32, IF], dt)
        nc.vector.tensor_tensor(out=itf, in0=ps_itf, in1=bif_b, op=ALU.add)

        wv = itf[:, 0:128]
        ersig = wk.tile([32, 128], dt)
        nc.scalar.activation(out=ersig, in_=itf[:, 128:256], func=AF.Sigmoid)
        wgag = wk.tile([32, 2], dt)
        nc.scalar.activation(out=wgag, in_=itf[:, 256:258], func=AF.Sigmoid)
        wg = wgag[:, 0:1]
        agt = wgag[:, 1:2]
        expm = wk.tile([32, 12], dt)
        nc.scalar.activation(out=expm, in_=itf[:, 259:271], func=AF.Exp)
        msum = wk.tile([32, 4], dt)
        nc.vector.tensor_reduce(out=msum,
                                in_=expm.rearrange("p (r k) -> p r k", k=3),
                                axis=mybir.AxisListType.X, op=ALU.add)
        minv = wk.tile([32, 4], dt)
        nc.vector.reciprocal(out=minv, in_=msum)
        sc16 = wk.tile([32, 16], dt)   # [rstr | m0 | m1 | m2]
        nc.scalar.activation(out=sc16[:, 0:4], in_=itf[:, 271:275],
                             func=AF.Exp)
        nc.scalar.activation(out=sc16[:, 0:4], in_=sc16[:, 0:4],
                             func=AF.Ln, bias=1.0)
        em3 = expm.rearrange("p (r k) -> p r k", k=3)
        for kk in range(3):
            nc.vector.tensor_mul(out=sc16[:, 4 + 4 * kk:8 + 4 * kk],
                                 in0=em3[:, :, kk], in1=minv)
        ps_t16 = ppt.tile([16, 32], dt, tag="tr")
        nc.tensor.transpose(ps_t16, sc16, ident[0:32, 0:32])
        t16 = wk.tile([16, 32], dt)
        nc.vector.tensor_copy(out=t16, in_=ps_t16)
        cols4 = wk.tile([128, 4], dt)  # [str | m0 | m1 | m2] as rb-columns
        for q in range(4):
            nc.sync.dma_start(out=cols4[:, q:q + 1],
                              in_=t16[4 * q:4 * q + 4, :])
        str_col = cols4[:, 0:1]
        m0_col = cols4[:, 1:2]
        m1_col = cols4[:, 2:3]
        m2_col = cols4[:, 3:4]

        ev = wk.tile([32, 128], dt)
        nc.vector.tensor_scalar(out=ev, in0=ersig, scalar1=wg, scalar2=None,
                                op0=ALU.mult)
        av = wk.tile([32, 128], dt)
        nc.vector.tensor_scalar(out=av, in0=wv, scalar1=wg, scalar2=None,
                                op0=ALU.mult)

        sq = wk.tile([32, 128], dt, tag="sq")
        nrm = wk.tile([32, 1], dt, tag="nrm")
        nc.scalar.activation(out=sq, in_=wv, func=AF.Square, accum_out=nrm)
        nc.scalar.activation(out=nrm, in_=nrm, func=AF.Sqrt)
        nc.vector.tensor_scalar(out=nrm, in0=nrm, scalar1=1e-12, scalar2=None,
                                op0=ALU.max)
        nc.vector.reciprocal(out=nrm, in_=nrm)
        nwv = wk.tile([32, 128], dt)
        nc.vector.tensor_scalar(out=nwv, in0=wv, scalar1=nrm, scalar2=None,
                                op0=ALU.mult)
        ps_nwvT = ppt.tile([128, 32], dt, tag="tr")
        nc.tensor.transpose(ps_nwvT, nwv, ident[0:32, 0:32])
        nwvT = wk.tile([128, 32], dt)
        nc.vector.tensor_copy(out=nwvT, in_=ps_nwvT)

        # ============ Stage D: write addressing + partials -> AllReduce =====
        memnT = wk.tile([128, N], dt, tag="memnT", bufs=1)
        for t in range(NT):
            mt = mem_sb[:, t * 128:(t + 1) * 128]
            sqm = wk.tile([128, 128], dt, tag="sqm", bufs=2)
            nrmc = wk.tile([128, 1], dt, tag="nrmc", bufs=2)
            nc.scalar.activation(out=sqm, in_=mt, func=AF.Square, accum_out=nrmc)
            nc.scalar.activation(out=nrmc, in_=nrmc, func=AF.Sqrt)
            nc.vector.tensor_scalar(out=nrmc, in0=nrmc, scalar1=1e-12,
                                    scalar2=None, op0=ALU.max)
            nc.vector.reciprocal(out=nrmc, in_=nrmc)
            nc.vector.tensor_scalar(out=sqm, in0=mt, scalar1=nrmc, scalar2=None,
                                    op0=ALU.mult)
            ps_tr = ppt.tile([128, 128], dt, tag="tr")
            nc.tensor.transpose(ps_tr, sqm, ident)
            nc.vector.tensor_copy(out=memnT[:, t * 128:(t + 1) * 128], in_=ps_tr)

        ps_cw = ppb.tile([32, N], dt, tag="big")
        for ch in range(4):
            nc.tensor.matmul(ps_cw[:, ch * 512:(ch + 1) * 512], lhsT=nwvT,
                             rhs=memnT[:, ch * 512:(ch + 1) * 512],
                             start=True, stop=True)
        cwexp = wk.tile([32, N], dt)
        den = wk.tile([32, 1], dt)
        nc.scalar.activation(out=cwexp, in_=ps_cw, func=AF.Exp, accum_out=den)
        nc.vector.reciprocal(out=den, in_=den)
        a_sc = wk.tile([32, 1], dt)
        nc.vector.tensor_mul(out=a_sc, in0=wg, in1=den)
        nc.vector.tensor_scalar(out=a_sc, in0=a_sc, scalar1=0.5, scalar2=None,
                                op0=ALU.mult)
        b_sc = wk.tile([32, 1], dt)
        nc.vector.tensor_mul(out=b_sc, in0=wg, in1=agt)
        nc.vector.tensor_scalar(out=b_sc, in0=b_sc, scalar1=0.5, scalar2=None,
                                op0=ALU.mult)
        wt = cwexp
        nc.vector.tensor_scalar(out=wt, in0=cwexp, scalar1=a_sc, scalar2=None,
                                op0=ALU.mult)
        alloc_b = wk.tile([32, N], dt, tag="allocb", bufs=1)
        nc.sync.dma_start(out=alloc_b,
                          in_=ag_out[:, 0:256].partition_broadcast(32))
        nc.vector.scalar_tensor_tensor(out=wt, in0=alloc_b,
                                       scalar=b_sc, in1=wt, op0=ALU.mult,
                                       op1=ALU.add)
        wsq = wk.tile([32, N], dt)
        nc.vector.tensor_mul(out=wsq, in0=wt, in1=wt)
        s_col = wk.tile([32, 1], dt)
        nc.vector.tensor_reduce(out=s_col, in_=wt, axis=mybir.AxisListType.X,
                                op=ALU.add)
        rhs_eva = wk.tile([32, 257], dt)
        nc.vector.tensor_copy(out=rhs_eva[:, 0:128], in_=ev)
        nc.vector.tensor_copy(out=rhs_eva[:, 128:256], in_=av)
        nc.vector.tensor_copy(out=rhs_eva[:, 256:257], in_=s_col)
        for t in range(NT):
            ps_p = ppt.tile([128, 257], dt, tag="tr")
            nc.tensor.matmul(ps_p, lhsT=wt[:, t * 128:(t + 1) * 128],
                             rhs=rhs_eva, start=True, stop=True)
            ps_q = ppt.tile([128, 1], dt, tag="trq", bufs=1)
            nc.tensor.matmul(ps_q, lhsT=wsq[:, t * 128:(t + 1) * 128],
                             rhs=ones_col[0:32, :], start=True, stop=True)
            arsb = wk.tile([128, 256], dt, tag="arsb", bufs=2)
            nc.vector.tensor_copy(out=arsb, in_=ps_p[:, 0:256])
            lucol = wk.tile([128, 1], dt, tag="lucol", bufs=2)
            qsb = wk.tile([128, 1], dt, tag="qsb", bufs=2)
            nc.vector.tensor_copy(out=qsb, in_=ps_q)
            nc.vector.tensor_sub(out=lucol, in0=ps_p[:, 256:257], in1=qsb)
            nc.sync.dma_start(out=arA_in[t], in_=arsb)
            nc.sync.dma_start(
                out=ar_in[ARA + N + t * 128:ARA + N + (t + 1) * 128], in_=lucol)

        nc.gpsimd.collective_compute(
            "AllReduce", ALU.add, replica_groups=[list(range(M))],
            ins=[ar_in[:]], outs=[ar_out[:]])

        # ============ Stage E: memory update + read weights ============
        # broadcast csum/lus/rowsum from DRAM into (128,N) tiles, then
        # BW_b = 0.9/N*csum + 0.1/(N*B)*lus ; FW_b same with rowsum.
        BW_b = wk.tile([128, N], dt, tag="usgb", bufs=1)
        lus_b = wk.tile([128, N], dt, tag="lb", bufs=1)
        FW_b = wk.tile([128, N], dt, tag="step", bufs=1)
        nc.sync.dma_start(out=BW_b,
                          in_=ar_out[ARA:ARA + N].partition_broadcast(128))
        nc.sync.dma_start(out=lus_b,
                          in_=ar_out[ARA + N:ARA + 2 * N].partition_broadcast(128))
        nc.sync.dma_start(out=FW_b,
                          in_=ag_out[:, 256:512].partition_broadcast(128))
        nc.vector.tensor_scalar(out=lus_b, in0=lus_b, scalar1=0.1 / (N * B),
                                scalar2=None, op0=ALU.mult)
        nc.vector.scalar_tensor_tensor(out=BW_b, in0=BW_b, scalar=0.9 / N,
                                       in1=lus_b, op0=ALU.mult, op1=ALU.add)
        nc.vector.scalar_tensor_tensor(out=FW_b, in0=FW_b, scalar=0.9 / N,
                                       in1=lus_b, op0=ALU.mult, op1=ALU.add)

        mnew = wk.tile([128, N], dt)
        mnewT = wk.tile([128, N], dt, tag="memnT", bufs=1)
        for t in range(NT):
            ea = wk.tile([128, 256], dt, tag="arsb", bufs=2)
            nc.sync.dma_start(out=ea, in_=arA_out[t])
            f1 = wk.tile([128, 128], dt, tag="f1", bufs=2)
            nc.vector.tensor_scalar(out=f1, in0=ea[:, 0:128], scalar1=-1.0 / B,
                                    scalar2=1.0, op0=ALU.mult, op1=ALU.add)
            nc.vector.tensor_mul(out=f1, in0=f1,
                                 in1=mem_sb[:, t * 128:(t + 1) * 128])
            nc.vector.scalar_tensor_tensor(out=mnew[:, t * 128:(t + 1) * 128],
                                           in0=ea[:, 128:256], scalar=1.0 / B,
                                           in1=f1, op0=ALU.mult, op1=ALU.add)
            mt = mnew[:, t * 128:(t + 1) * 128]
            sqm = wk.tile([128, 128], dt, tag="sqm", bufs=2)
            nrmc = wk.tile([128, 1], dt, tag="nrmc", bufs=2)
            nc.scalar.activation(out=sqm, in_=mt, func=AF.Square, accum_out=nrmc)
            nc.scalar.activation(out=nrmc, in_=nrmc, func=AF.Sqrt)
            nc.vector.tensor_scalar(out=nrmc, in0=nrmc, scalar1=1e-12,
                                    scalar2=None, op0=ALU.max)
            nc.vector.reciprocal(out=nrmc, in_=nrmc)
            nc.vector.tensor_scalar(out=sqm, in0=mt, scalar1=nrmc, scalar2=None,
                                    op0=ALU.mult)
            ps_tr = ppt.tile([128, 128], dt, tag="tr")
            nc.tensor.transpose(ps_tr, sqm, ident)
            nc.vector.tensor_copy(out=mnewT[:, t * 128:(t + 1) * 128], in_=ps_tr)

        nkT = wk.tile([128, 128], dt)
        for r in range(R):
            rk = itf[:, 275 + 128 * r:275 + 128 * (r + 1)]
            sqk = wk.tile([32, 128], dt, tag="sqk", bufs=2)
            nrk = wk.tile([32, 1], dt, tag="nrk", bufs=2)
            nc.scalar.activation(out=sqk, in_=rk, func=AF.Square, accum_out=nrk)
            nc.scalar.activation(out=nrk, in_=nrk, func=AF.Sqrt)
            nc.vector.tensor_scalar(out=nrk, in0=nrk, scalar1=1e-12,
                                    scalar2=None, op0=ALU.max)
            nc.vector.reciprocal(out=nrk, in_=nrk)
            nc.vector.tensor_scalar(out=sqk, in0=rk, scalar1=nrk, scalar2=None,
                                    op0=ALU.mult)
            ps_k = ppt.tile([128, 32], dt, tag="tr")
            nc.tensor.transpose(ps_k, sqk, ident[0:32, 0:32])
            nc.vector.tensor_copy(out=nkT[:, r * 32:(r + 1) * 32], in_=ps_k)

        ps_sim = ppb.tile([128, N], dt, tag="big")
        for ch in range(4):
            nc.tensor.matmul(ps_sim[:, ch * 512:(ch + 1) * 512], lhsT=nkT,
                             rhs=mnewT[:, ch * 512:(ch + 1) * 512],
                             start=True, stop=True)
        esim = wk.tile([128, N], dt)
        dsum = wk.tile([128, 1], dt)
        nc.scalar.activation(out=esim, in_=ps_sim, func=AF.Exp, scale=str_col,
                             accum_out=dsum)
        nc.vector.reciprocal(out=dsum, in_=dsum)
        c0 = wk.tile([128, 1], dt)
        nc.vector.tensor_mul(out=c0, in0=m0_col, in1=dsum)
        nrw = esim
        nc.vector.tensor_scalar(out=nrw, in0=esim, scalar1=c0, scalar2=None,
                                op0=ALU.mult)
        nc.vector.scalar_tensor_tensor(out=nrw, in0=BW_b,
                                       scalar=m1_col, in1=nrw, op0=ALU.mult,
                                       op1=ALU.add)
        nc.vector.scalar_tensor_tensor(out=nrw, in0=FW_b,
                                       scalar=m2_col, in1=nrw, op0=ALU.mult,
                                       op1=ALU.add)

        ps_ro = pp1.tile([128, 128], dt, tag="psA")
        roT = wk.tile([128, 128], dt)
        for t in range(NT):
            ps_tr = ppt.tile([128, 128], dt, tag="tr")
            nc.tensor.transpose(ps_tr, nrw[:, t * 128:(t + 1) * 128], ident)
            nrwT = wk.tile([128, 128], dt, tag="nrwT", bufs=2)
            nc.vector.tensor_copy(out=nrwT, in_=ps_tr)
            nc.tensor.matmul(ps_ro, lhsT=mnew[:, t * 128:(t + 1) * 128],
                             rhs=nrwT, start=(t == 0), stop=(t == NT - 1))
        nc.vector.tensor_copy(out=roT, in_=ps_ro)

        if DEBUG:
            nc.sync.dma_start(out=d_cin[:, :], in_=cin.rearrange("p k b -> p (k b)"))
            nc.sync.dma_start(out=d_cmean[:, :], in_=cmean)
            nc.sync.dma_start(out=d_hT[:, :], in_=hT.rearrange("p k b -> p (k b)"))
            nc.sync.dma_start(out=d_itf[:, :], in_=itf)
            nc.sync.dma_start(out=d_r0[:, :], in_=r0)
            nc.sync.dma_start(out=d_mnew[:, :], in_=mnew)
            nc.sync.dma_start(out=d_wt[:, :], in_=wt)
            nc.sync.dma_start(out=d_cols[:, :], in_=cols4)
            nc.sync.dma_start(out=d_bw[:, :], in_=BW_b[0:1, :])
            nc.sync.dma_start(out=d_fw[:, :], in_=FW_b[0:1, :])
            nc.sync.dma_start(out=d_nrw[:, :], in_=nrw)
            nc.sync.dma_start(out=d_roT[:, :], in_=roT)
        # ============ Stage F: output projection ============
        for o in range(8):
            ps_o = ppt.tile([128, BL], dt, tag="tr")
            for k in range(KC):
                wok = wstr.tile([128, 128], dt, tag="wok")
                nc.sync.dma_start(
                    out=wok, in_=wout[:, k * 1024 + o * 128:k * 1024 + (o + 1) * 128])
                rhs = hT[:, k, :] if k < KH else \
                    roT[:, (k - KH) * 32:(k - KH + 1) * 32]
                nc.tensor.matmul(ps_o, lhsT=wok, rhs=rhs,
                                 start=(k == 0), stop=(k == KC - 1))
            oc = wk.tile([128, BL], dt, tag="oc", bufs=2)
            nc.scalar.activation(out=oc, in_=ps_o, func=AF.Identity,
                                 bias=boc_sb[:, o:o + 1])
            nc.sync.dma_start(out=outT[o], in_=oc)

    nc.finalize()
    return nc


def _prep_inputs(x, memory, usage, link, W_ih, W_hh, b_ih, b_hh, W_if, b_if,
                 W_out, b_out):
    f = np.float32
    x = np.asarray(x, f); memory = np.asarray(memory, f)
    usage = np.asarray(usage, f); link = np.asarray(link, f)
    W_ih = np.asarray(W_ih, f); b_ih = np.asarray(b_ih, f)
    b_hh = np.asarray(b_hh, f); W_if = np.asarray(W_if, f)
    b_if = np.asarray(b_if, f); W_out = np.asarray(W_out, f)
    b_out = np.asarray(b_out, f)

    sel = np.r_[0:1024, 2048:4096]
    W3T = W_ih[sel].T                             # (1536, 3072)
    w3 = np.ascontiguousarray(
        W3T.reshape(KC, 128, 24, 128).transpose(1, 2, 0, 3).reshape(128, KC * 3072))
    b3 = (b_ih + b_hh)[sel]
    b3c = np.ascontiguousarray(b3.reshape(24, 128).T)
    wif = np.ascontiguousarray(
        W_if.T.reshape(KH, 128, IF).transpose(1, 0, 2).reshape(128, KH * IF))
    wout = np.ascontiguousarray(
        W_out.T.reshape(KC, 128, 1024).transpose(1, 0, 2).reshape(128, KC * 1024))
    boc = np.ascontiguousarray(b_out.reshape(8, 128).T)
    memA = np.ascontiguousarray(
        memory.reshape(NT, 128, 128).transpose(1, 0, 2).reshape(128, N))
    bifr = b_if.reshape(1, IF)
    usg = usage.reshape(1, N)

    shared = dict(w3=w3, b3c=b3c, wif=wif, bifr=bifr, wout=wout, boc=boc,
                  memA=memA, usg=usg)
    in_maps = []
    for c in range(M):
        xs = x[c * BL:(c + 1) * BL]               # (32, 1024)
        cinx = np.ascontiguousarray(
            xs.T.reshape(KH, 128, BL).transpose(1, 0, 2).reshape(128, KH * BL))
        ls = link[c * 256:(c + 1) * 256]          # (256, 2048)
        lnkm = np.ascontiguousarray(
            ls.reshape(2, 128, N).transpose(1, 0, 2).reshape(128, 2 * N))
        ucols = np.ascontiguousarray(
            usage.reshape(NT, 128)[2 * c:2 * c + 2].T)      # (128, 2)
        m = dict(shared)
        m["cinx"] = cinx
        m["lnk"] = lnkm
        m["ucols"] = ucols
        in_maps.append(m)
    return in_maps


def kernel(**inputs):
    nc = build_nc()
    in_maps = _prep_inputs(**inputs)
    res = run_bass_kernel_spmd(nc, in_maps, list(range(M))).results
    outs = []
    for c in range(M):
        oT = res[c]["outT"]                       # (8, 128, 32)
        outs.append(np.transpose(oT, (2, 0, 1)).reshape(BL, 1024))
    return np.concatenate(outs, 0).astype(np.float32)


# revision 20
# speedup vs baseline: 1.4668x; 1.4668x over previous
"""DNC single-step forward on 8 Trainium2 NeuronCores (Bass/Tile).

Data-parallel over batch (B=256 -> 32/core); memory/link/usage/params
replicated. Cross-core collectives:
  - one AllGather: [allocation-weight shard | link row-sum shard]
  - one AllReduce: [erase/add matrix partials | link col-sum partial |
    masked-lu sum partial]

Math restructuring (validated vs reference at ~1e-6 in numpy):
  - h0=c0=0  =>  W_hh and the forget gate are dead.
  - prev_rw uniform 1/N  =>  read_flat = tiled col-mean(memory); the
    (B,R,N)@(N,N) backward/forward einsums collapse to (1/N)*col/row-sums
    of link_new.
  - lu = ww^T ww / B is symmetric; its diag-masked row/col sums reduce to
    (w^T s - sum_b w^2)/B with s = row-sums of w; N x N lu never formed.
  - allocation weights sort-free: alloc[i] = u[i]*exp(sum_{u_k<u_i} ln(1-u_k)).
"""
import sys

sys.path.insert(0, '/opt/trn_rl_repo')

import numpy as np
import ml_dtypes
import concourse.bass as bass
import concourse.bacc as bacc
import concourse.tile as tile
from concourse import mybir
from concourse.bass_utils import run_bass_kernel_spmd
from concourse.masks import make_identity

AF = mybir.ActivationFunctionType
ALU = mybir.AluOpType
F32 = mybir.dt.float32
BF16 = mybir.dt.bfloat16

B, N, D, R, H, I = 256, 2048, 128, 4, 1024, 1024
CI = I + R * D          # 1536
IF = 787
M = 8                   # cores
BL = B // M             # 32 batch rows per core
NT = N // 128           # 16 n-tiles
KC = CI // 128          # 12 k-tiles of cin
KH = H // 128           # 8 k-tiles of h

ARA = NT * 128 * 256    # AR region A: [ep|ap] per n-tile
ARTOT = ARA + 2 * N     # + region B: [link colsum | lu stat]
DEBUG = False

# (1,N) vectors live at quarter partition offsets of two (128,N) tiles
# r0: usg@0, L@32, alloc@64, rowsum@96 ; r1: csum@0, lus@32, BW@64, FW@96


def build_nc():
    nc = bacc.Bacc("TRN2", target_bir_lowering=False, num_devices=M)
    dt = F32
    # ---- inputs (host-prepared layouts; per-partition contiguous DMA) ----
    cinx = nc.declare_dram_parameter("cinx", [128, KH * BL], BF16, isOutput=False)
    w3 = nc.declare_dram_parameter("w3", [128, KC * 3072], BF16, isOutput=False)
    b3c = nc.declare_dram_parameter("b3c", [128, 24], dt, isOutput=False)
    wif = nc.declare_dram_parameter("wif", [128, KH * IF], BF16, isOutput=False)
    bifr = nc.declare_dram_parameter("bifr", [1, IF], dt, isOutput=False)
    wout = nc.declare_dram_parameter("wout", [128, KC * 1024], BF16, isOutput=False)
    boc = nc.declare_dram_parameter("boc", [128, 8], dt, isOutput=False)
    memA = nc.declare_dram_parameter("memA", [128, N], dt, isOutput=False)
    lnk = nc.declare_dram_parameter("lnk", [128, 2 * N], dt, isOutput=False)
    usg = nc.declare_dram_parameter("usg", [1, N], dt, isOutput=False)
    ucols = nc.declare_dram_parameter("ucols", [128, 2], dt, isOutput=False)

    outT = nc.declare_dram_parameter("outT", [8, 128, BL], dt, isOutput=True)
    if DEBUG:
        d_hT = nc.declare_dram_parameter("d_hT", [128, KH * BL], dt, isOutput=True)
        d_itf = nc.declare_dram_parameter("d_itf", [32, IF], dt, isOutput=True)
        d_r0 = nc.declare_dram_parameter("d_r0", [128, N], dt, isOutput=True)
        d_mnew = nc.declare_dram_parameter("d_mnew", [128, N], dt, isOutput=True)
        d_wt = nc.declare_dram_parameter("d_wt", [32, N], dt, isOutput=True)
        d_cols = nc.declare_dram_parameter("d_cols", [128, 4], dt, isOutput=True)
        d_bw = nc.declare_dram_parameter("d_bw", [1, N], dt, isOutput=True)
        d_fw = nc.declare_dram_parameter("d_fw", [1, N], dt, isOutput=True)
        d_nrw = nc.declare_dram_parameter("d_nrw", [128, N], dt, isOutput=True)
        d_roT = nc.declare_dram_parameter("d_roT", [128, 128], dt, isOutput=True)
        d_cin = nc.declare_dram_parameter("d_cin", [128, KH * BL], dt, isOutput=True)
        d_cmean = nc.declare_dram_parameter("d_cmean", [128, BL], dt, isOutput=True)


    from contextlib import ExitStack
    with tile.TileContext(nc) as tc, ExitStack() as es:
        cons = es.enter_context(tc.tile_pool(name="cons", bufs=1))
        wk = es.enter_context(tc.tile_pool(name="wk", bufs=1))
        wstr = es.enter_context(tc.tile_pool(name="wstr", bufs=2))
        lstr = es.enter_context(tc.tile_pool(name="lstr", bufs=1))
        dpool = es.enter_context(tc.tile_pool(name="dram", bufs=1, space="DRAM"))
        pp1 = es.enter_context(tc.tile_pool(name="pp1", bufs=1, space="PSUM"))
        ppb = es.enter_context(tc.tile_pool(name="ppb", bufs=1, space="PSUM"))
        ppt = es.enter_context(tc.tile_pool(name="ppt", bufs=2, space="PSUM"))

        ag_in = dpool.tile([512], dt)
        ag_out = dpool.tile([M, 512], dt, addr_space="Shared")
        ar_in = dpool.tile([ARTOT], dt)
        ar_out = dpool.tile([ARTOT], dt, addr_space="Shared")
        arA_in = ar_in[0:ARA].rearrange("(t p f) -> t p f", p=128, f=256)
        arA_out = ar_out[0:ARA].rearrange("(t p f) -> t p f", p=128, f=256)

        ident = cons.tile([128, 128], dt)
        make_identity(nc, ident)
        ones_col = cons.tile([128, 1], dt)
        nc.vector.memset(ones_col, 1.0)
        ones32 = cons.tile([128, BL], dt)
        nc.vector.memset(ones32, 1.0)

        r0 = cons.tile([128, N], dt)
        r1 = cons.tile([128, N], dt)
        nc.sync.dma_start(out=r0[0:1, :], in_=usg[:, :])
        bif_sb = cons.tile([1, IF], dt)
        nc.sync.dma_start(out=bif_sb, in_=bifr[:, :])
        b3_sb = cons.tile([128, 24], dt)
        nc.sync.dma_start(out=b3_sb, in_=b3c[:, :])
        boc_sb = cons.tile([128, 8], dt)
        nc.sync.dma_start(out=boc_sb, in_=boc[:, :])
        mem_sb = cons.tile([128, N], dt)
        nc.sync.dma_start(out=mem_sb, in_=memA[:, :])
        uc_sb = cons.tile([128, 2], dt)
        nc.sync.dma_start(out=uc_sb, in_=ucols[:, :])

        usg_row = r0[0:1, :]
        tmp_row = r0[0:1, :]  # reused in stage E (usage dead by then)

        # ============ Stage A: alloc shard + link stats -> AllGather ========
        usg_b = wk.tile([128, N], dt, tag="usgb")
        nc.sync.dma_start(out=usg_b, in_=usg[0:1, :].partition_broadcast(128))
        L_b = wk.tile([128, N], dt, tag="lb")
        nc.scalar.activation(out=L_b, in_=usg_b, func=AF.Ln, bias=1.0,
                             scale=-1.0)
        for i in range(2):
            u_col = uc_sb[:, i:i + 1]
            step = wk.tile([128, N], dt, tag="step", bufs=1)
            nc.vector.tensor_scalar(out=step, in0=usg_b,
                                    scalar1=u_col, scalar2=None, op0=ALU.is_lt)
            nc.vector.tensor_tensor(out=step, in0=step, in1=L_b, op=ALU.mult)
            a_col = wk.tile([128, 1], dt, tag="acol", bufs=2)
            nc.vector.tensor_reduce(out=a_col, in_=step,
                                    axis=mybir.AxisListType.X, op=ALU.add)
            nc.scalar.activation(out=a_col, in_=a_col, func=AF.Exp)
            nc.vector.tensor_mul(out=a_col, in0=a_col, in1=u_col)
            nc.sync.dma_start(out=ag_in[i * 128:(i + 1) * 128], in_=a_col)

        cs_ps = ppb.tile([1, N], dt, tag="big")
        for i in range(2):
            lt = lstr.tile([128, N], dt, tag="lnk")
            nc.sync.dma_start(out=lt, in_=lnk[:, i * N:(i + 1) * N])
            rs = wk.tile([128, 1], dt, tag="rs", bufs=2)
            nc.vector.tensor_reduce(out=rs, in_=lt, axis=mybir.AxisListType.X,
                                    op=ALU.add)
            nc.sync.dma_start(out=ag_in[256 + i * 128:256 + (i + 1) * 128],
                              in_=rs)
            for ch in range(4):
                nc.tensor.matmul(cs_ps[0:1, ch * 512:(ch + 1) * 512],
                                 lhsT=ones_col,
                                 rhs=lt[:, ch * 512:(ch + 1) * 512],
                                 start=(i == 0), stop=(i == 1))
        cs_row = wk.tile([1, N], dt, tag="csrow")
        nc.scalar.copy(out=cs_row, in_=cs_ps)
        nc.sync.dma_start(out=ar_in[ARA:ARA + N], in_=cs_row)

        nc.gpsimd.collective_compute(
            "AllGather", ALU.bypass, replica_groups=[list(range(M))],
            ins=[ag_in[:]], outs=[ag_out.flatten()])
        nc.sync.dma_start(out=r0[64:65, :], in_=ag_out[:, 0:256])
        nc.sync.dma_start(out=r0[96:97, :], in_=ag_out[:, 256:512])
        alloc_row = r0[64:65, :]
        rowsum_row = r0[96:97, :]

        # ============ Stage B: LSTM ============
        cin = wk.tile([128, KH, BL], BF16)
        nc.sync.dma_start(out=cin,
                          in_=cinx[:, :].rearrange("p (k b) -> p k b", b=BL))
        mean_ps = ppt.tile([1, 128], dt, tag="tr")
        for t in range(NT):
            nc.tensor.matmul(mean_ps, lhsT=ones_col,
                             rhs=mem_sb[:, t * 128:(t + 1) * 128],
                             start=(t == 0), stop=(t == NT - 1))
        mean_row = wk.tile([1, 128], dt, tag="meanr")
        nc.scalar.activation(out=mean_row, in_=mean_ps, func=AF.Copy,
                             scale=1.0 / N)
        mc_ps = ppt.tile([128, 1], dt, tag="tr")
        nc.tensor.transpose(mc_ps, mean_row, ident[0:1, 0:1])
        mean_col = wk.tile([128, 1], dt, tag="meanc")
        nc.vector.tensor_copy(out=mean_col, in_=mc_ps)
        cmean = wk.tile([128, BL], BF16)
        nc.scalar.activation(out=cmean, in_=ones32, func=AF.Copy,
                             scale=mean_col)

        # j-outer / k-inner: one psum bank per accumulation group (a
        # start=True matmul claims a whole 2KB zero-region, so slices of one
        # bank cannot host interleaved groups).
        gsb = wk.tile([128, 24, BL], dt)   # activated gates, j-major
        for j in range(24):
            w3j = wstr.tile([128, KC * 128], BF16, tag="w3j")
            nc.sync.dma_start(out=w3j,
                              in_=w3[:, j * KC * 128:(j + 1) * KC * 128])
            ps_j = ppt.tile([128, BL], dt, tag="tr")
            for k in range(KC):
                rhs_k = cin[:, k, :] if k < KH else cmean
                nc.tensor.matmul(ps_j, lhsT=w3j[:, k * 128:(k + 1) * 128],
                                 rhs=rhs_k,
                                 start=(k == 0), stop=(k == KC - 1))
            fn = AF.Tanh if 8 <= j < 16 else AF.Sigmoid
            nc.scalar.activation(out=gsb[:, j, :], in_=ps_j, func=fn,
                                 bias=b3_sb[:, j:j + 1])
        hT = wk.tile([128, KH, BL], BF16)
        for t in range(KH):
            cc = wk.tile([128, BL], dt, tag="g1", bufs=2)
            nc.vector.tensor_mul(out=cc, in0=gsb[:, t, :], in1=gsb[:, 8 + t, :])
            nc.scalar.activation(out=cc, in_=cc, func=AF.Tanh)
            nc.vector.tensor_mul(out=hT[:, t, :], in0=cc, in1=gsb[:, 16 + t, :])

        # ============ Stage C: interface vector ============
        ps_itf = ppb.tile([32, IF], dt, tag="big")
        for k in range(KH):
            wfk = wstr.tile([128, IF], BF16, tag="wifk")
            nc.sync.dma_start(out=wfk, in_=wif[:, k * IF:(k + 1) * IF])
            nc.tensor.matmul(ps_itf[:, 0:512], lhsT=hT[:, k, :],
                             rhs=wfk[:, 0:512], start=(k == 0),
                             stop=(k == KH - 1))
            nc.tensor.matmul(ps_itf[:, 512:IF], lhsT=hT[:, k, :],
                             rhs=wfk[:, 512:IF], start=(k == 0),
                             stop=(k == KH - 1))
        bif_b = wk.tile([32, IF], dt)
        nc.sync.dma_start(out=bif_b, in_=bifr[0:1, :].partition_broadcast(32))
        itf = wk.tile([32, IF], dt)
        nc.vector.tensor_tensor(out=itf, in0=ps_itf, in1=bif_b, op=ALU.add)

        wv = itf[:, 0:128]
        ersig = wk.tile([32, 128], dt)
        nc.scalar.activation(out=ersig, in_=itf[:, 128:256], func=AF.Sigmoid)
        wgag = wk.tile([32, 2], dt)
        nc.scalar.activation(out=wgag, in_=itf[:, 256:258], func=AF.Sigmoid)
        wg = wgag[:, 0:1]
        agt = wgag[:, 1:2]
        expm = wk.tile([32, 12], dt)
        nc.scalar.activation(out=expm, in_=itf[:, 259:271], func=AF.Exp)
        msum = wk.tile([32, 4], dt)
        nc.vector.tensor_reduce(out=msum,
                                in_=expm.rearrange("p (r k) -> p r k", k=3),
                                axis=mybir.AxisListType.X, op=ALU.add)
        minv = wk.tile([32, 4], dt)
        nc.vector.reciprocal(out=minv, in_=msum)
        sc16 = wk.tile([32, 16], dt)   # [rstr | m0 | m1 | m2]
        nc.scalar.activation(out=sc16[:, 0:4], in_=itf[:, 271:275],
                             func=AF.Exp)
        nc.scalar.activation(out=sc16[:, 0:4], in_=sc16[:, 0:4],
                             func=AF.Ln, bias=1.0)
        em3 = expm.rearrange("p (r k) -> p r k", k=3)
        for kk in range(3):
            nc.vector.tensor_mul(out=sc16[:, 4 + 4 * kk:8 + 4 * kk],
                                 in0=em3[:, :, kk], in1=minv)
        ps_t16 = ppt.tile([16, 32], dt, tag="tr")
        nc.tensor.transpose(ps_t16, sc16, ident[0:32, 0:32])
        t16 = wk.tile([16, 32], dt)
        nc.vector.tensor_copy(out=t16, in_=ps_t16)
        cols4 = wk.tile([128, 4], dt)  # [str | m0 | m1 | m2] as rb-columns
        for q in range(4):
            nc.sync.dma_start(out=cols4[:, q:q + 1],
                              in_=t16[4 * q:4 * q + 4, :])
        str_col = cols4[:, 0:1]
        m0_col = cols4[:, 1:2]
        m1_col = cols4[:, 2:3]
        m2_col = cols4[:, 3:4]

        ev = wk.tile([32, 128], dt)
        nc.vector.tensor_scalar(out=ev, in0=ersig, scalar1=wg, scalar2=None,
                                op0=ALU.mult)
        av = wk.tile([32, 128], dt)
        nc.vector.tensor_scalar(out=av, in0=wv, scalar1=wg, scalar2=None,
                                op0=ALU.mult)

        sq = wk.tile([32, 128], dt, tag="sq")
        nrm = wk.tile([32, 1], dt, tag="nrm")
        nc.scalar.activation(out=sq, in_=wv, func=AF.Square, accum_out=nrm)
        nc.scalar.activation(out=nrm, in_=nrm, func=AF.Sqrt)
        nc.vector.tensor_scalar(out=nrm, in0=nrm, scalar1=1e-12, scalar2=None,
                                op0=ALU.max)
        nc.vector.reciprocal(out=nrm, in_=nrm)
        nwv = wk.tile([32, 128], dt)
        nc.vector.tensor_scalar(out=nwv, in0=wv, scalar1=nrm, scalar2=None,
                                op0=ALU.mult)
        ps_nwvT = ppt.tile([128, 32], dt, tag="tr")
        nc.tensor.transpose(ps_nwvT, nwv, ident[0:32, 0:32])
        nwvT = wk.tile([128, 32], dt)
        nc.vector.tensor_copy(out=nwvT, in_=ps_nwvT)

        # ============ Stage D: write addressing + partials -> AllReduce =====
        memnT = wk.tile([128, N], dt, tag="memnT", bufs=1)
        for t in range(NT):
            mt = mem_sb[:, t * 128:(t + 1) * 128]
            sqm = wk.tile([128, 128], dt, tag="sqm", bufs=2)
            nrmc = wk.tile([128, 1], dt, tag="nrmc", bufs=2)
            nc.scalar.activation(out=sqm, in_=mt, func=AF.Square, accum_out=nrmc)
            nc.scalar.activation(out=nrmc, in_=nrmc, func=AF.Sqrt)
            nc.vector.tensor_scalar(out=nrmc, in0=nrmc, scalar1=1e-12,
                                    scalar2=None, op0=ALU.max)
            nc.vector.reciprocal(out=nrmc, in_=nrmc)
            nc.vector.tensor_scalar(out=sqm, in0=mt, scalar1=nrmc, scalar2=None,
                                    op0=ALU.mult)
            ps_tr = ppt.tile([128, 128], dt, tag="tr")
            nc.tensor.transpose(ps_tr, sqm, ident)
            nc.vector.tensor_copy(out=memnT[:, t * 128:(t + 1) * 128], in_=ps_tr)

        ps_cw = ppb.tile([32, N], dt, tag="big")
        for ch in range(4):
            nc.tensor.matmul(ps_cw[:, ch * 512:(ch + 1) * 512], lhsT=nwvT,
                             rhs=memnT[:, ch * 512:(ch + 1) * 512],
                             start=True, stop=True)
        cwexp = wk.tile([32, N], dt)
        den = wk.tile([32, 1], dt)
        nc.scalar.activation(out=cwexp, in_=ps_cw, func=AF.Exp, accum_out=den)
        nc.vector.reciprocal(out=den, in_=den)
        a_sc = wk.tile([32, 1], dt)
        nc.vector.tensor_mul(out=a_sc, in0=wg, in1=den)
        nc.vector.tensor_scalar(out=a_sc, in0=a_sc, scalar1=0.5, scalar2=None,
                                op0=ALU.mult)
        b_sc = wk.tile([32, 1], dt)
        nc.vector.tensor_mul(out=b_sc, in0=wg, in1=agt)
        nc.vector.tensor_scalar(out=b_sc, in0=b_sc, scalar1=0.5, scalar2=None,
                                op0=ALU.mult)
        wt = cwexp
        nc.vector.tensor_scalar(out=wt, in0=cwexp, scalar1=a_sc, scalar2=None,
                                op0=ALU.mult)
        alloc_b = wk.tile([32, N], dt, tag="allocb", bufs=1)
        nc.sync.dma_start(out=alloc_b,
                          in_=ag_out[:, 0:256].partition_broadcast(32))
        nc.vector.scalar_tensor_tensor(out=wt, in0=alloc_b,
                                       scalar=b_sc, in1=wt, op0=ALU.mult,
                                       op1=ALU.add)
        wsq = wk.tile([32, N], dt)
        nc.vector.tensor_mul(out=wsq, in0=wt, in1=wt)
        s_col = wk.tile([32, 1], dt)
        nc.vector.tensor_reduce(out=s_col, in_=wt, axis=mybir.AxisListType.X,
                                op=ALU.add)
        rhs_eva = wk.tile([32, 257], dt)
        nc.vector.tensor_copy(out=rhs_eva[:, 0:128], in_=ev)
        nc.vector.tensor_copy(out=rhs_eva[:, 128:256], in_=av)
        nc.vector.tensor_copy(out=rhs_eva[:, 256:257], in_=s_col)
        for t in range(NT):
            ps_p = ppt.tile([128, 257], dt, tag="tr")
            nc.tensor.matmul(ps_p, lhsT=wt[:, t * 128:(t + 1) * 128],
                             rhs=rhs_eva, start=True, stop=True)
            ps_q = ppt.tile([128, 1], dt, tag="trq", bufs=1)
            nc.tensor.matmul(ps_q, lhsT=wsq[:, t * 128:(t + 1) * 128],
                             rhs=ones_col[0:32, :], start=True, stop=True)
            arsb = wk.tile([128, 256], dt, tag="arsb", bufs=2)
            nc.vector.tensor_copy(out=arsb, in_=ps_p[:, 0:256])
            lucol = wk.tile([128, 1], dt, tag="lucol", bufs=2)
            qsb = wk.tile([128, 1], dt, tag="qsb", bufs=2)
            nc.vector.tensor_copy(out=qsb, in_=ps_q)
            nc.vector.tensor_sub(out=lucol, in0=ps_p[:, 256:257], in1=qsb)
            nc.sync.dma_start(out=arA_in[t], in_=arsb)
            nc.sync.dma_start(
                out=ar_in[ARA + N + t * 128:ARA + N + (t + 1) * 128], in_=lucol)

        nc.gpsimd.collective_compute(
            "AllReduce", ALU.add, replica_groups=[list(range(M))],
            ins=[ar_in[:]], outs=[ar_out[:]])

        # ============ Stage E: memory update + read weights ============
        # broadcast csum/lus/rowsum from DRAM into (128,N) tiles, then
        # BW_b = 0.9/N*csum + 0.1/(N*B)*lus ; FW_b same with rowsum.
        BW_b = wk.tile([128, N], dt, tag="usgb", bufs=1)
        lus_b = wk.tile([128, N], dt, tag="lb", bufs=1)
        FW_b = wk.tile([128, N], dt, tag="step", bufs=1)
        nc.sync.dma_start(out=BW_b,
                          in_=ar_out[ARA:ARA + N].partition_broadcast(128))
        nc.sync.dma_start(out=lus_b,
                          in_=ar_out[ARA + N:ARA + 2 * N].partition_broadcast(128))
        nc.sync.dma_start(out=FW_b,
                          in_=ag_out[:, 256:512].partition_broadcast(128))
        nc.vector.tensor_scalar(out=lus_b, in0=lus_b, scalar1=0.1 / (N * B),
                                scalar2=None, op0=ALU.mult)
        nc.vector.scalar_tensor_tensor(out=BW_b, in0=BW_b, scalar=0.9 / N,
                                       in1=lus_b, op0=ALU.mult, op1=ALU.add)
        nc.vector.scalar_tensor_tensor(out=FW_b, in0=FW_b, scalar=0.9 / N,
                                       in1=lus_b, op0=ALU.mult, op1=ALU.add)

        mnew = wk.tile([128, N], dt)
        mnewT = wk.tile([128, N], dt, tag="memnT", bufs=1)
        for t in range(NT):
            ea = wk.tile([128, 256], dt, tag="arsb", bufs=2)
            nc.sync.dma_start(out=ea, in_=arA_out[t])
            f1 = wk.tile([128, 128], dt, tag="f1", bufs=2)
            nc.vector.tensor_scalar(out=f1, in0=ea[:, 0:128], scalar1=-1.0 / B,
                                    scalar2=1.0, op0=ALU.mult, op1=ALU.add)
            nc.vector.tensor_mul(out=f1, in0=f1,
                                 in1=mem_sb[:, t * 128:(t + 1) * 128])
            nc.vector.scalar_tensor_tensor(out=mnew[:, t * 128:(t + 1) * 128],
                                           in0=ea[:, 128:256], scalar=1.0 / B,
                                           in1=f1, op0=ALU.mult, op1=ALU.add)
            mt = mnew[:, t * 128:(t + 1) * 128]
            sqm = wk.tile([128, 128], dt, tag="sqm", bufs=2)
            nrmc = wk.tile([128, 1], dt, tag="nrmc", bufs=2)
            nc.scalar.activation(out=sqm, in_=mt, func=AF.Square, accum_out=nrmc)
            nc.scalar.activation(out=nrmc, in_=nrmc, func=AF.Sqrt)
            nc.vector.tensor_scalar(out=nrmc, in0=nrmc, scalar1=1e-12,
                                    scalar2=None, op0=ALU.max)
            nc.vector.reciprocal(out=nrmc, in_=nrmc)
            nc.vector.tensor_scalar(out=sqm, in0=mt, scalar1=nrmc, scalar2=None,
                                    op0=ALU.mult)
            ps_tr = ppt.tile([128, 128], dt, tag="tr")
            nc.tensor.transpose(ps_tr, sqm, ident)
            nc.vector.tensor_copy(out=mnewT[:, t * 128:(t + 1) * 128], in_=ps_tr)

        nkT = wk.tile([128, 128], dt)
        for r in range(R):
            rk = itf[:, 275 + 128 * r:275 + 128 * (r + 1)]
            sqk = wk.tile([32, 128], dt, tag="sqk", bufs=2)
            nrk = wk.tile([32, 1], dt, tag="nrk", bufs=2)
            nc.scalar.activation(out=sqk, in_=rk, func=AF.Square, accum_out=nrk)
            nc.scalar.activation(out=nrk, in_=nrk, func=AF.Sqrt)
            nc.vector.tensor_scalar(out=nrk, in0=nrk, scalar1=1e-12,
                                    scalar2=None, op0=ALU.max)
            nc.vector.reciprocal(out=nrk, in_=nrk)
            nc.vector.tensor_scalar(out=sqk, in0=rk, scalar1=nrk, scalar2=None,
                                    op0=ALU.mult)
            ps_k = ppt.tile([128, 32], dt, tag="tr")
            nc.tensor.transpose(ps_k, sqk, ident[0:32, 0:32])
            nc.vector.tensor_copy(out=nkT[:, r * 32:(r + 1) * 32], in_=ps_k)

        ps_sim = ppb.tile([128, N], dt, tag="big")
        for ch in range(4):
            nc.tensor.matmul(ps_sim[:, ch * 512:(ch + 1) * 512], lhsT=nkT,
                             rhs=mnewT[:, ch * 512:(ch + 1) * 512],
                             start=True, stop=True)
        esim = wk.tile([128, N], dt)
        dsum = wk.tile([128, 1], dt)
        nc.scalar.activation(out=esim, in_=ps_sim, func=AF.Exp, scale=str_col,
                             accum_out=dsum)
        nc.vector.reciprocal(out=dsum, in_=dsum)
        c0 = wk.tile([128, 1], dt)
        nc.vector.tensor_mul(out=c0, in0=m0_col, in1=dsum)
        nrw = esim
        nc.vector.tensor_scalar(out=nrw, in0=esim, scalar1=c0, scalar2=None,
                                op0=ALU.mult)
        nc.vector.scalar_tensor_tensor(out=nrw, in0=BW_b,
                                       scalar=m1_col, in1=nrw, op0=ALU.mult,
                                       op1=ALU.add)
        nc.vector.scalar_tensor_tensor(out=nrw, in0=FW_b,
                                       scalar=m2_col, in1=nrw, op0=ALU.mult,
                                       op1=ALU.add)

        ps_ro = pp1.tile([128, 128], dt, tag="psA")
        roT = wk.tile([128, 128], BF16)
        for t in range(NT):
            ps_tr = ppt.tile([128, 128], dt, tag="tr")
            nc.tensor.transpose(ps_tr, nrw[:, t * 128:(t + 1) * 128], ident)
            nrwT = wk.tile([128, 128], dt, tag="nrwT", bufs=2)
            nc.vector.tensor_copy(out=nrwT, in_=ps_tr)
            nc.tensor.matmul(ps_ro, lhsT=mnew[:, t * 128:(t + 1) * 128],
                             rhs=nrwT, start=(t == 0), stop=(t == NT - 1))
        nc.vector.tensor_copy(out=roT, in_=ps_ro)

        if DEBUG:
            nc.sync.dma_start(out=d_cin[:, :], in_=cin.rearrange("p k b -> p (k b)"))
            nc.sync.dma_start(out=d_cmean[:, :], in_=cmean)
            nc.sync.dma_start(out=d_hT[:, :], in_=hT.rearrange("p k b -> p (k b)"))
            nc.sync.dma_start(out=d_itf[:, :], in_=itf)
            nc.sync.dma_start(out=d_r0[:, :], in_=r0)
            nc.sync.dma_start(out=d_mnew[:, :], in_=mnew)
            nc.sync.dma_start(out=d_wt[:, :], in_=wt)
            nc.sync.dma_start(out=d_cols[:, :], in_=cols4)
            nc.sync.dma_start(out=d_bw[:, :], in_=BW_b[0:1, :])
            nc.sync.dma_start(out=d_fw[:, :], in_=FW_b[0:1, :])
            nc.sync.dma_start(out=d_nrw[:, :], in_=nrw)
            nc.sync.dma_start(out=d_roT[:, :], in_=roT)
        # ============ Stage F: output projection ============
        wout_t = []
        for k in range(KC):
            wt_k = cons.tile([128, 1024], BF16, name=f"wout{k}")
            nc.sync.dma_start(out=wt_k, in_=wout[:, k * 1024:(k + 1) * 1024])
            wout_t.append(wt_k)
        for o in range(8):
            ps_o = ppt.tile([128, BL], dt, tag="tr")
            for k in range(KC):
                rhs = hT[:, k, :] if k < KH else \
                    roT[:, (k - KH) * 32:(k - KH + 1) * 32]
                nc.tensor.matmul(ps_o, lhsT=wout_t[k][:, o * 128:(o + 1) * 128],
                                 rhs=rhs, start=(k == 0), stop=(k == KC - 1))
            oc = wk.tile([128, BL], dt, tag="oc", bufs=2)
            nc.scalar.activation(out=oc, in_=ps_o, func=AF.Identity,
                                 bias=boc_sb[:, o:o + 1])
            nc.sync.dma_start(out=outT[o], in_=oc)

    nc.finalize()
    return nc


def _prep_inputs(x, memory, usage, link, W_ih, W_hh, b_ih, b_hh, W_if, b_if,
                 W_out, b_out):
    f = np.float32
    x = np.asarray(x, f); memory = np.asarray(memory, f)
    usage = np.asarray(usage, f); link = np.asarray(link, f)
    W_ih = np.asarray(W_ih, f); b_ih = np.asarray(b_ih, f)
    b_hh = np.asarray(b_hh, f); W_if = np.asarray(W_if, f)
    b_if = np.asarray(b_if, f); W_out = np.asarray(W_out, f)
    b_out = np.asarray(b_out, f)

    sel = np.r_[0:1024, 2048:4096]
    W3T = W_ih[sel].T                             # (1536, 3072)
    w3 = np.ascontiguousarray(
        W3T.reshape(KC, 128, 24, 128).transpose(1, 2, 0, 3)
        .reshape(128, KC * 3072).astype(ml_dtypes.bfloat16))
    b3 = (b_ih + b_hh)[sel]
    b3c = np.ascontiguousarray(b3.reshape(24, 128).T)
    wif = np.ascontiguousarray(
        W_if.T.reshape(KH, 128, IF).transpose(1, 0, 2)
        .reshape(128, KH * IF).astype(ml_dtypes.bfloat16))
    wout = np.ascontiguousarray(
        W_out.T.reshape(KC, 128, 1024).transpose(1, 0, 2)
        .reshape(128, KC * 1024).astype(ml_dtypes.bfloat16))
    boc = np.ascontiguousarray(b_out.reshape(8, 128).T)
    memA = np.ascontiguousarray(
        memory.reshape(NT, 128, 128).transpose(1, 0, 2).reshape(128, N))
    bifr = b_if.reshape(1, IF)
    usg = usage.reshape(1, N)

    shared = dict(w3=w3, b3c=b3c, wif=wif, bifr=bifr, wout=wout, boc=boc,
                  memA=memA, usg=usg)
    in_maps = []
    for c in range(M):
        xs = x[c * BL:(c + 1) * BL]               # (32, 1024)
        cinx = np.ascontiguousarray(
            xs.T.reshape(KH, 128, BL).transpose(1, 0, 2)
            .reshape(128, KH * BL).astype(ml_dtypes.bfloat16))
        ls = link[c * 256:(c + 1) * 256]          # (256, 2048)
        lnkm = np.ascontiguousarray(
            ls.reshape(2, 128, N).transpose(1, 0, 2).reshape(128, 2 * N))
        ucols = np.ascontiguousarray(
            usage.reshape(NT, 128)[2 * c:2 * c + 2].T)      # (128, 2)
        m = dict(shared)
        m["cinx"] = cinx
        m["lnk"] = lnkm
        m["ucols"] = ucols
        in_maps.append(m)
    return in_maps


def kernel(**inputs):
    nc = build_nc()
    in_maps = _prep_inputs(**inputs)
    res = run_bass_kernel_spmd(nc, in_maps, list(range(M))).results
    outs = []
    for c in range(M):
        oT = res[c]["outT"]                       # (8, 128, 32)
        outs.append(np.transpose(oT, (2, 0, 1)).reshape(BL, 1024))
    return np.concatenate(outs, 0).astype(np.float32)


# revision 21
# speedup vs baseline: 1.8998x; 1.2952x over previous
"""DNC single-step forward on 8 Trainium2 NeuronCores (Bass/Tile).

Data-parallel over batch (B=256 -> 32/core); memory/link/usage/params
replicated. Cross-core collectives:
  - one AllGather: [allocation-weight shard | link row-sum shard]
  - one AllReduce: [erase/add matrix partials | link col-sum partial |
    masked-lu sum partial]

Math restructuring (validated vs reference at ~1e-6 in numpy):
  - h0=c0=0  =>  W_hh and the forget gate are dead.
  - prev_rw uniform 1/N  =>  read_flat = tiled col-mean(memory); the
    (B,R,N)@(N,N) backward/forward einsums collapse to (1/N)*col/row-sums
    of link_new.
  - lu = ww^T ww / B is symmetric; its diag-masked row/col sums reduce to
    (w^T s - sum_b w^2)/B with s = row-sums of w; N x N lu never formed.
  - allocation weights sort-free: alloc[i] = u[i]*exp(sum_{u_k<u_i} ln(1-u_k)).
"""
import sys

sys.path.insert(0, '/opt/trn_rl_repo')

import numpy as np
import ml_dtypes
import concourse.bass as bass
import concourse.bacc as bacc
import concourse.tile as tile
from concourse import mybir
from concourse.bass_utils import run_bass_kernel_spmd
from concourse.masks import make_identity

AF = mybir.ActivationFunctionType
ALU = mybir.AluOpType
F32 = mybir.dt.float32
BF16 = mybir.dt.bfloat16

B, N, D, R, H, I = 256, 2048, 128, 4, 1024, 1024
CI = I + R * D          # 1536
IF = 787
M = 8                   # cores
BL = B // M             # 32 batch rows per core
NT = N // 128           # 16 n-tiles
KC = CI // 128          # 12 k-tiles of cin
KH = H // 128           # 8 k-tiles of h

ARA = NT * 128 * 256    # AR region A: [ep|ap] per n-tile
ARTOT = ARA + 2 * N     # + region B: [link colsum | lu stat]
DEBUG = False

# (1,N) vectors live at quarter partition offsets of two (128,N) tiles
# r0: usg@0, L@32, alloc@64, rowsum@96 ; r1: csum@0, lus@32, BW@64, FW@96


def build_nc():
    nc = bacc.Bacc("TRN2", target_bir_lowering=False, num_devices=M)
    dt = F32
    # ---- inputs (host-prepared layouts; per-partition contiguous DMA) ----
    cinx = nc.declare_dram_parameter("cinx", [128, KH * BL], BF16, isOutput=False)
    w3 = nc.declare_dram_parameter("w3", [128, KC * 3072], BF16, isOutput=False)
    b3c = nc.declare_dram_parameter("b3c", [128, 24], dt, isOutput=False)
    wif = nc.declare_dram_parameter("wif", [128, KH * IF], BF16, isOutput=False)
    bifr = nc.declare_dram_parameter("bifr", [1, IF], dt, isOutput=False)
    wout = nc.declare_dram_parameter("wout", [128, KC * 1024], BF16, isOutput=False)
    boc = nc.declare_dram_parameter("boc", [128, 8], dt, isOutput=False)
    memA = nc.declare_dram_parameter("memA", [128, N], dt, isOutput=False)
    lnk = nc.declare_dram_parameter("lnk", [128, 2 * N], dt, isOutput=False)
    usg = nc.declare_dram_parameter("usg", [1, N], dt, isOutput=False)
    ucols = nc.declare_dram_parameter("ucols", [128, 2], dt, isOutput=False)

    outT = nc.declare_dram_parameter("outT", [8, 128, BL], dt, isOutput=True)
    if DEBUG:
        d_hT = nc.declare_dram_parameter("d_hT", [128, KH * BL], dt, isOutput=True)
        d_itf = nc.declare_dram_parameter("d_itf", [32, IF], dt, isOutput=True)
        d_r0 = nc.declare_dram_parameter("d_r0", [128, N], dt, isOutput=True)
        d_mnew = nc.declare_dram_parameter("d_mnew", [128, N], dt, isOutput=True)
        d_wt = nc.declare_dram_parameter("d_wt", [32, N], dt, isOutput=True)
        d_cols = nc.declare_dram_parameter("d_cols", [128, 4], dt, isOutput=True)
        d_bw = nc.declare_dram_parameter("d_bw", [1, N], dt, isOutput=True)
        d_fw = nc.declare_dram_parameter("d_fw", [1, N], dt, isOutput=True)
        d_nrw = nc.declare_dram_parameter("d_nrw", [128, N], dt, isOutput=True)
        d_roT = nc.declare_dram_parameter("d_roT", [128, 128], dt, isOutput=True)
        d_cin = nc.declare_dram_parameter("d_cin", [128, KH * BL], dt, isOutput=True)
        d_cmean = nc.declare_dram_parameter("d_cmean", [128, BL], dt, isOutput=True)


    from contextlib import ExitStack
    with tile.TileContext(nc) as tc, ExitStack() as es:
        cons = es.enter_context(tc.tile_pool(name="cons", bufs=1))
        wk = es.enter_context(tc.tile_pool(name="wk", bufs=1))
        wstr = es.enter_context(tc.tile_pool(name="wstr", bufs=2))
        lstr = es.enter_context(tc.tile_pool(name="lstr", bufs=1))
        dpool = es.enter_context(tc.tile_pool(name="dram", bufs=1, space="DRAM"))
        pp1 = es.enter_context(tc.tile_pool(name="pp1", bufs=1, space="PSUM"))
        ppb = es.enter_context(tc.tile_pool(name="ppb", bufs=1, space="PSUM"))
        ppt = es.enter_context(tc.tile_pool(name="ppt", bufs=2, space="PSUM"))

        ag_in = dpool.tile([512], dt)
        ag_out = dpool.tile([M, 512], dt, addr_space="Shared")
        ar_in = dpool.tile([ARTOT], BF16)
        ar_out = dpool.tile([ARTOT], BF16, addr_space="Shared")
        arA_in = ar_in[0:ARA].rearrange("(t p f) -> t p f", p=128, f=256)
        arA_out = ar_out[0:ARA].rearrange("(t p f) -> t p f", p=128, f=256)

        ident = cons.tile([128, 128], dt)
        make_identity(nc, ident)
        ones_col = cons.tile([128, 1], dt)
        nc.vector.memset(ones_col, 1.0)
        ones32 = cons.tile([128, BL], dt)
        nc.vector.memset(ones32, 1.0)

        r0 = cons.tile([128, N], dt)
        r1 = cons.tile([128, N], dt)
        nc.sync.dma_start(out=r0[0:1, :], in_=usg[:, :])
        bif_sb = cons.tile([1, IF], dt)
        nc.sync.dma_start(out=bif_sb, in_=bifr[:, :])
        b3_sb = cons.tile([128, 24], dt)
        nc.sync.dma_start(out=b3_sb, in_=b3c[:, :])
        boc_sb = cons.tile([128, 8], dt)
        nc.sync.dma_start(out=boc_sb, in_=boc[:, :])
        mem_sb = cons.tile([128, N], dt)
        nc.sync.dma_start(out=mem_sb, in_=memA[:, :])
        uc_sb = cons.tile([128, 2], dt)
        nc.sync.dma_start(out=uc_sb, in_=ucols[:, :])

        usg_row = r0[0:1, :]
        tmp_row = r0[0:1, :]  # reused in stage E (usage dead by then)

        # ============ Stage A: alloc shard + link stats -> AllGather ========
        ones_row = cons.tile([1, 128], dt)
        nc.vector.memset(ones_row, 1.0)
        usg_b = wk.tile([128, N], dt, tag="usgb")
        bc_ps = ppb.tile([128, N], dt, tag="big")
        for ch in range(4):
            nc.tensor.matmul(bc_ps[:, ch * 512:(ch + 1) * 512], lhsT=ones_row,
                             rhs=usg_row[:, ch * 512:(ch + 1) * 512],
                             start=True, stop=True)
        nc.vector.tensor_copy(out=usg_b, in_=bc_ps)
        L_b = wk.tile([128, N], dt, tag="lb")
        nc.scalar.activation(out=L_b, in_=usg_b, func=AF.Ln, bias=1.0,
                             scale=-1.0)
        for i in range(2):
            u_col = uc_sb[:, i:i + 1]
            step = wk.tile([128, N], dt, tag="step", bufs=1)
            nc.vector.tensor_scalar(out=step, in0=usg_b,
                                    scalar1=u_col, scalar2=None, op0=ALU.is_lt)
            nc.vector.tensor_tensor(out=step, in0=step, in1=L_b, op=ALU.mult)
            a_col = wk.tile([128, 1], dt, tag="acol", bufs=2)
            nc.vector.tensor_reduce(out=a_col, in_=step,
                                    axis=mybir.AxisListType.X, op=ALU.add)
            nc.scalar.activation(out=a_col, in_=a_col, func=AF.Exp)
            nc.vector.tensor_mul(out=a_col, in0=a_col, in1=u_col)
            nc.sync.dma_start(out=ag_in[i * 128:(i + 1) * 128], in_=a_col)

        cs_ps = ppb.tile([1, N], dt, tag="big")
        for i in range(2):
            lt = lstr.tile([128, N], dt, tag="lnk")
            nc.sync.dma_start(out=lt, in_=lnk[:, i * N:(i + 1) * N])
            rs = wk.tile([128, 1], dt, tag="rs", bufs=2)
            nc.vector.tensor_reduce(out=rs, in_=lt, axis=mybir.AxisListType.X,
                                    op=ALU.add)
            nc.sync.dma_start(out=ag_in[256 + i * 128:256 + (i + 1) * 128],
                              in_=rs)
            for ch in range(4):
                nc.tensor.matmul(cs_ps[0:1, ch * 512:(ch + 1) * 512],
                                 lhsT=ones_col,
                                 rhs=lt[:, ch * 512:(ch + 1) * 512],
                                 start=(i == 0), stop=(i == 1))
        cs_row = wk.tile([1, N], BF16, tag="csrow")
        nc.scalar.copy(out=cs_row, in_=cs_ps)
        nc.sync.dma_start(out=ar_in[ARA:ARA + N], in_=cs_row)

        nc.gpsimd.collective_compute(
            "AllGather", ALU.bypass, replica_groups=[list(range(M))],
            ins=[ag_in[:]], outs=[ag_out.flatten()])
        alloc_r = cons.tile([1, N], dt)
        nc.sync.dma_start(out=alloc_r, in_=ag_out[:, 0:256])

        # ============ Stage B: LSTM ============
        cin = wk.tile([128, KH, BL], BF16)
        nc.sync.dma_start(out=cin,
                          in_=cinx[:, :].rearrange("p (k b) -> p k b", b=BL))
        mean_ps = ppt.tile([1, 128], dt, tag="tr")
        for t in range(NT):
            nc.tensor.matmul(mean_ps, lhsT=ones_col,
                             rhs=mem_sb[:, t * 128:(t + 1) * 128],
                             start=(t == 0), stop=(t == NT - 1))
        mean_row = wk.tile([1, 128], dt, tag="meanr")
        nc.scalar.activation(out=mean_row, in_=mean_ps, func=AF.Copy,
                             scale=1.0 / N)
        mc_ps = ppt.tile([128, 1], dt, tag="tr")
        nc.tensor.transpose(mc_ps, mean_row, ident[0:1, 0:1])
        mean_col = wk.tile([128, 1], dt, tag="meanc")
        nc.vector.tensor_copy(out=mean_col, in_=mc_ps)
        cmean = wk.tile([128, BL], BF16)
        nc.scalar.activation(out=cmean, in_=ones32, func=AF.Copy,
                             scale=mean_col)

        # j-outer / k-inner: one psum bank per accumulation group (a
        # start=True matmul claims a whole 2KB zero-region, so slices of one
        # bank cannot host interleaved groups).
        gsb = wk.tile([128, 24, BL], dt)   # activated gates, j-major
        for j in range(24):
            w3j = wstr.tile([128, KC * 128], BF16, tag="w3j", bufs=4)
            nc.sync.dma_start(out=w3j,
                              in_=w3[:, j * KC * 128:(j + 1) * KC * 128])
            ps_j = ppt.tile([128, BL], dt, tag="tr")
            for k in range(KC):
                rhs_k = cin[:, k, :] if k < KH else cmean
                nc.tensor.matmul(ps_j, lhsT=w3j[:, k * 128:(k + 1) * 128],
                                 rhs=rhs_k,
                                 start=(k == 0), stop=(k == KC - 1))
            fn = AF.Tanh if 8 <= j < 16 else AF.Sigmoid
            nc.scalar.activation(out=gsb[:, j, :], in_=ps_j, func=fn,
                                 bias=b3_sb[:, j:j + 1])
        hT = wk.tile([128, KH, BL], BF16)
        for t in range(KH):
            cc = wk.tile([128, BL], dt, tag="g1", bufs=2)
            nc.vector.tensor_mul(out=cc, in0=gsb[:, t, :], in1=gsb[:, 8 + t, :])
            nc.scalar.activation(out=cc, in_=cc, func=AF.Tanh)
            nc.vector.tensor_mul(out=hT[:, t, :], in0=cc, in1=gsb[:, 16 + t, :])

        # ============ Stage C: interface vector ============
        ps_itf = ppb.tile([32, IF], dt, tag="big")
        for k in range(KH):
            wfk = wstr.tile([128, IF], BF16, tag="wifk")
            nc.sync.dma_start(out=wfk, in_=wif[:, k * IF:(k + 1) * IF])
            nc.tensor.matmul(ps_itf[:, 0:512], lhsT=hT[:, k, :],
                             rhs=wfk[:, 0:512], start=(k == 0),
                             stop=(k == KH - 1))
            nc.tensor.matmul(ps_itf[:, 512:IF], lhsT=hT[:, k, :],
                             rhs=wfk[:, 512:IF], start=(k == 0),
                             stop=(k == KH - 1))
        bif_b = wk.tile([32, IF], dt)
        nc.sync.dma_start(out=bif_b, in_=bifr[0:1, :].partition_broadcast(32))
        itf = wk.tile([32, IF], dt)
        nc.vector.tensor_tensor(out=itf, in0=ps_itf, in1=bif_b, op=ALU.add)

        wv = itf[:, 0:128]
        ersig = wk.tile([32, 128], dt)
        nc.scalar.activation(out=ersig, in_=itf[:, 128:256], func=AF.Sigmoid)
        wgag = wk.tile([32, 2], dt)
        nc.scalar.activation(out=wgag, in_=itf[:, 256:258], func=AF.Sigmoid)
        wg = wgag[:, 0:1]
        agt = wgag[:, 1:2]
        expm = wk.tile([32, 12], dt)
        nc.scalar.activation(out=expm, in_=itf[:, 259:271], func=AF.Exp)
        msum = wk.tile([32, 4], dt)
        nc.vector.tensor_reduce(out=msum,
                                in_=expm.rearrange("p (r k) -> p r k", k=3),
                                axis=mybir.AxisListType.X, op=ALU.add)
        minv = wk.tile([32, 4], dt)
        nc.vector.reciprocal(out=minv, in_=msum)
        sc16 = wk.tile([32, 16], dt)   # [rstr | m0 | m1 | m2]
        nc.scalar.activation(out=sc16[:, 0:4], in_=itf[:, 271:275],
                             func=AF.Exp)
        nc.scalar.activation(out=sc16[:, 0:4], in_=sc16[:, 0:4],
                             func=AF.Ln, bias=1.0)
        em3 = expm.rearrange("p (r k) -> p r k", k=3)
        for kk in range(3):
            nc.vector.tensor_mul(out=sc16[:, 4 + 4 * kk:8 + 4 * kk],
                                 in0=em3[:, :, kk], in1=minv)
        ps_t16 = ppt.tile([16, 32], dt, tag="tr")
        nc.tensor.transpose(ps_t16, sc16, ident[0:32, 0:32])
        t16 = wk.tile([16, 32], dt)
        nc.vector.tensor_copy(out=t16, in_=ps_t16)
        cols4 = wk.tile([128, 4], dt)  # [str | m0 | m1 | m2] as rb-columns
        for q in range(4):
            nc.sync.dma_start(out=cols4[:, q:q + 1],
                              in_=t16[4 * q:4 * q + 4, :])
        str_col = cols4[:, 0:1]
        m0_col = cols4[:, 1:2]
        m1_col = cols4[:, 2:3]
        m2_col = cols4[:, 3:4]

        ev = wk.tile([32, 128], dt)
        nc.vector.tensor_scalar(out=ev, in0=ersig, scalar1=wg, scalar2=None,
                                op0=ALU.mult)
        av = wk.tile([32, 128], dt)
        nc.vector.tensor_scalar(out=av, in0=wv, scalar1=wg, scalar2=None,
                                op0=ALU.mult)

        sq = wk.tile([32, 128], dt, tag="sq")
        nrm = wk.tile([32, 1], dt, tag="nrm")
        nc.scalar.activation(out=sq, in_=wv, func=AF.Square, accum_out=nrm)
        nc.scalar.activation(out=nrm, in_=nrm, func=AF.Sqrt)
        nc.vector.tensor_scalar(out=nrm, in0=nrm, scalar1=1e-12, scalar2=None,
                                op0=ALU.max)
        nc.vector.reciprocal(out=nrm, in_=nrm)
        nwv = wk.tile([32, 128], dt)
        nc.vector.tensor_scalar(out=nwv, in0=wv, scalar1=nrm, scalar2=None,
                                op0=ALU.mult)
        ps_nwvT = ppt.tile([128, 32], dt, tag="tr")
        nc.tensor.transpose(ps_nwvT, nwv, ident[0:32, 0:32])
        nwvT = wk.tile([128, 32], dt)
        nc.vector.tensor_copy(out=nwvT, in_=ps_nwvT)

        # ============ Stage D: write addressing + partials -> AllReduce =====
        memnT = wk.tile([128, N], dt, tag="memnT", bufs=1)
        for t in range(NT):
            mt = mem_sb[:, t * 128:(t + 1) * 128]
            sqm = wk.tile([128, 128], dt, tag="sqm", bufs=2)
            nrmc = wk.tile([128, 1], dt, tag="nrmc", bufs=2)
            nc.scalar.activation(out=sqm, in_=mt, func=AF.Square, accum_out=nrmc)
            nc.scalar.activation(out=nrmc, in_=nrmc, func=AF.Sqrt)
            nc.vector.tensor_scalar(out=nrmc, in0=nrmc, scalar1=1e-12,
                                    scalar2=None, op0=ALU.max)
            nc.vector.reciprocal(out=nrmc, in_=nrmc)
            nc.vector.tensor_scalar(out=sqm, in0=mt, scalar1=nrmc, scalar2=None,
                                    op0=ALU.mult)
            ps_tr = ppt.tile([128, 128], dt, tag="tr")
            nc.tensor.transpose(ps_tr, sqm, ident)
            nc.vector.tensor_copy(out=memnT[:, t * 128:(t + 1) * 128], in_=ps_tr)

        ps_cw = ppb.tile([32, N], dt, tag="big")
        for ch in range(4):
            nc.tensor.matmul(ps_cw[:, ch * 512:(ch + 1) * 512], lhsT=nwvT,
                             rhs=memnT[:, ch * 512:(ch + 1) * 512],
                             start=True, stop=True)
        cwexp = wk.tile([32, N], dt)
        den = wk.tile([32, 1], dt)
        nc.scalar.activation(out=cwexp, in_=ps_cw, func=AF.Exp, accum_out=den)
        nc.vector.reciprocal(out=den, in_=den)
        a_sc = wk.tile([32, 1], dt)
        nc.vector.tensor_mul(out=a_sc, in0=wg, in1=den)
        nc.vector.tensor_scalar(out=a_sc, in0=a_sc, scalar1=0.5, scalar2=None,
                                op0=ALU.mult)
        b_sc = wk.tile([32, 1], dt)
        nc.vector.tensor_mul(out=b_sc, in0=wg, in1=agt)
        nc.vector.tensor_scalar(out=b_sc, in0=b_sc, scalar1=0.5, scalar2=None,
                                op0=ALU.mult)
        ps_bt = ppt.tile([1, 32], dt, tag="trq", bufs=1)
        nc.tensor.transpose(ps_bt, b_sc, ident[0:32, 0:32])
        b_scT = wk.tile([1, 32], dt)
        nc.vector.tensor_copy(out=b_scT, in_=ps_bt)
        ps_w = ppb.tile([32, N], dt, tag="big")
        for ch in range(4):
            nc.tensor.matmul(ps_w[:, ch * 512:(ch + 1) * 512], lhsT=b_scT,
                             rhs=alloc_r[:, ch * 512:(ch + 1) * 512],
                             start=True, stop=True)
        wt = cwexp
        for ch in range(4):
            nc.vector.scalar_tensor_tensor(
                out=wt[:, ch * 512:(ch + 1) * 512],
                in0=cwexp[:, ch * 512:(ch + 1) * 512], scalar=a_sc,
                in1=ps_w[:, ch * 512:(ch + 1) * 512], op0=ALU.mult, op1=ALU.add)
        wsq = wk.tile([32, N], dt)
        nc.vector.tensor_mul(out=wsq, in0=wt, in1=wt)
        s_col = wk.tile([32, 1], dt)
        nc.vector.tensor_reduce(out=s_col, in_=wt, axis=mybir.AxisListType.X,
                                op=ALU.add)
        rhs_eva = wk.tile([32, 257], dt)
        nc.vector.tensor_copy(out=rhs_eva[:, 0:128], in_=ev)
        nc.vector.tensor_copy(out=rhs_eva[:, 128:256], in_=av)
        nc.vector.tensor_copy(out=rhs_eva[:, 256:257], in_=s_col)
        stag = wk.tile([128, NT, 256], BF16)
        lustag = wk.tile([128, NT], BF16)
        for t in range(NT):
            ps_p = ppt.tile([128, 257], dt, tag="tr")
            nc.tensor.matmul(ps_p, lhsT=wt[:, t * 128:(t + 1) * 128],
                             rhs=rhs_eva, start=True, stop=True)
            ps_q = ppt.tile([128, 1], dt, tag="trq", bufs=1)
            nc.tensor.matmul(ps_q, lhsT=wsq[:, t * 128:(t + 1) * 128],
                             rhs=ones_col[0:32, :], start=True, stop=True)
            nc.vector.tensor_copy(out=stag[:, t, :], in_=ps_p[:, 0:256])
            qsb = wk.tile([128, 1], dt, tag="qsb", bufs=2)
            nc.vector.tensor_copy(out=qsb, in_=ps_q)
            nc.vector.tensor_sub(out=lustag[:, t:t + 1], in0=ps_p[:, 256:257],
                                 in1=qsb)
        nc.sync.dma_start(
            out=ar_in[0:ARA].rearrange("(t p f) -> p t f", p=128, f=256),
            in_=stag)
        nc.sync.dma_start(
            out=ar_in[ARA + N:ARA + 2 * N].rearrange("(t p) -> p t", p=128),
            in_=lustag)

        nc.gpsimd.collective_compute(
            "AllReduce", ALU.add, replica_groups=[list(range(M))],
            ins=[ar_in[:]], outs=[ar_out[:]])

        # ============ Stage E: memory update + read weights ============
        # [csum ; rowsum] rows -> BW/FW rows (2,N); lus row doubled to (2,N)
        cr2 = wk.tile([2, N], dt)
        csb = wk.tile([1, N], BF16, tag="csrow")
        nc.sync.dma_start(out=csb, in_=ar_out[ARA:ARA + N])
        nc.vector.tensor_copy(out=cr2[0:1, :], in_=csb)
        nc.sync.dma_start(out=cr2[1:2, :], in_=ag_out[:, 256:512])
        lu2 = wk.tile([2, N], BF16)
        nc.sync.dma_start(out=lu2[0:1, :], in_=ar_out[ARA + N:ARA + 2 * N])
        nc.sync.dma_start(out=lu2[1:2, :], in_=ar_out[ARA + N:ARA + 2 * N])
        bwfw = wk.tile([2, N], dt)
        nc.vector.tensor_scalar(out=bwfw, in0=cr2, scalar1=0.9 / N,
                                scalar2=None, op0=ALU.mult)
        nc.vector.scalar_tensor_tensor(out=bwfw, in0=lu2,
                                       scalar=0.1 / (N * B), in1=bwfw,
                                       op0=ALU.mult, op1=ALU.add)

        ea_full = wk.tile([128, NT, 256], BF16, tag="eaf")
        nc.sync.dma_start(
            out=ea_full,
            in_=ar_out[0:ARA].rearrange("(t p f) -> p t f", p=128, f=256))
        mnew = wk.tile([128, N], dt)
        mnew3 = mnew.rearrange("p (t d) -> p t d", d=128)
        f3 = wk.tile([128, NT, 128], dt, tag="step", bufs=1)
        nc.vector.tensor_scalar(out=f3, in0=ea_full[:, :, 0:128],
                                scalar1=-1.0 / B, scalar2=1.0, op0=ALU.mult,
                                op1=ALU.add)
        nc.vector.tensor_mul(out=f3, in0=f3,
                             in1=mem_sb.rearrange("p (t d) -> p t d", d=128))
        nc.vector.scalar_tensor_tensor(out=mnew3, in0=ea_full[:, :, 128:256],
                                       scalar=1.0 / B, in1=f3, op0=ALU.mult,
                                       op1=ALU.add)
        # batched row norms of mnew
        sqf = f3
        nc.vector.tensor_mul(out=sqf, in0=mnew3, in1=mnew3)
        nrm16 = wk.tile([128, NT], dt)
        nc.vector.tensor_reduce(out=nrm16, in_=sqf, axis=mybir.AxisListType.X,
                                op=ALU.add)
        nc.scalar.activation(out=nrm16, in_=nrm16, func=AF.Sqrt)
        nc.vector.tensor_scalar(out=nrm16, in0=nrm16, scalar1=1e-12,
                                scalar2=None, op0=ALU.max)
        nc.vector.reciprocal(out=nrm16, in_=nrm16)
        nmn = wk.tile([128, NT, 128], dt, tag="lb", bufs=1)
        nc.vector.tensor_tensor(out=nmn, in0=mnew3,
                                in1=nrm16.unsqueeze(2).broadcast_to([128, NT, 128]),
                                op=ALU.mult)
        mnewT = wk.tile([128, N], dt, tag="memnT", bufs=1)
        for t in range(NT):
            ps_tr = ppt.tile([128, 128], dt, tag="tr")
            nc.tensor.transpose(ps_tr, nmn[:, t, :], ident)
            nc.vector.tensor_copy(out=mnewT[:, t * 128:(t + 1) * 128], in_=ps_tr)

        nkT = wk.tile([128, 128], dt)
        for r in range(R):
            rk = itf[:, 275 + 128 * r:275 + 128 * (r + 1)]
            sqk = wk.tile([32, 128], dt, tag="sqk", bufs=2)
            nrk = wk.tile([32, 1], dt, tag="nrk", bufs=2)
            nc.scalar.activation(out=sqk, in_=rk, func=AF.Square, accum_out=nrk)
            nc.scalar.activation(out=nrk, in_=nrk, func=AF.Sqrt)
            nc.vector.tensor_scalar(out=nrk, in0=nrk, scalar1=1e-12,
                                    scalar2=None, op0=ALU.max)
            nc.vector.reciprocal(out=nrk, in_=nrk)
            nc.vector.tensor_scalar(out=sqk, in0=rk, scalar1=nrk, scalar2=None,
                                    op0=ALU.mult)
            ps_k = ppt.tile([128, 32], dt, tag="tr")
            nc.tensor.transpose(ps_k, sqk, ident[0:32, 0:32])
            nc.vector.tensor_copy(out=nkT[:, r * 32:(r + 1) * 32], in_=ps_k)

        ps_sim = ppb.tile([128, N], dt, tag="big")
        for ch in range(4):
            nc.tensor.matmul(ps_sim[:, ch * 512:(ch + 1) * 512], lhsT=nkT,
                             rhs=mnewT[:, ch * 512:(ch + 1) * 512],
                             start=True, stop=True)
        esim = wk.tile([128, N], dt)
        dsum = wk.tile([128, 1], dt)
        nc.scalar.activation(out=esim, in_=ps_sim, func=AF.Exp, scale=str_col,
                             accum_out=dsum)
        nc.vector.reciprocal(out=dsum, in_=dsum)
        c0 = wk.tile([128, 1], dt)
        nc.vector.tensor_mul(out=c0, in0=m0_col, in1=dsum)
        ps_mt = ppt.tile([2, 128], dt, tag="trq", bufs=1)
        nc.tensor.transpose(ps_mt, cols4[:, 2:4], ident)
        m12T = wk.tile([2, 128], dt)
        nc.vector.tensor_copy(out=m12T, in_=ps_mt)
        ps_term = ppb.tile([128, N], dt, tag="big")
        for ch in range(4):
            nc.tensor.matmul(ps_term[:, ch * 512:(ch + 1) * 512], lhsT=m12T,
                             rhs=bwfw[:, ch * 512:(ch + 1) * 512],
                             start=True, stop=True)
        nrw = esim
        for ch in range(4):
            nc.vector.scalar_tensor_tensor(
                out=nrw[:, ch * 512:(ch + 1) * 512],
                in0=esim[:, ch * 512:(ch + 1) * 512], scalar=c0,
                in1=ps_term[:, ch * 512:(ch + 1) * 512], op0=ALU.mult,
                op1=ALU.add)

        ps_ro = pp1.tile([128, 128], dt, tag="psA")
        roT = wk.tile([128, 128], BF16)
        for t in range(NT):
            ps_tr = ppt.tile([128, 128], dt, tag="tr")
            nc.tensor.transpose(ps_tr, nrw[:, t * 128:(t + 1) * 128], ident)
            nrwT = wk.tile([128, 128], dt, tag="nrwT", bufs=2)
            nc.vector.tensor_copy(out=nrwT, in_=ps_tr)
            nc.tensor.matmul(ps_ro, lhsT=mnew[:, t * 128:(t + 1) * 128],
                             rhs=nrwT, start=(t == 0), stop=(t == NT - 1))
        nc.vector.tensor_copy(out=roT, in_=ps_ro)

        if DEBUG:
            nc.sync.dma_start(out=d_cin[:, :], in_=cin.rearrange("p k b -> p (k b)"))
            nc.sync.dma_start(out=d_cmean[:, :], in_=cmean)
            nc.sync.dma_start(out=d_hT[:, :], in_=hT.rearrange("p k b -> p (k b)"))
            nc.sync.dma_start(out=d_itf[:, :], in_=itf)
            nc.sync.dma_start(out=d_r0[:, :], in_=r0)
            nc.sync.dma_start(out=d_mnew[:, :], in_=mnew)
            nc.sync.dma_start(out=d_wt[:, :], in_=wt)
            nc.sync.dma_start(out=d_cols[:, :], in_=cols4)
            nc.sync.dma_start(out=d_bw[:, :], in_=BW_b[0:1, :])
            nc.sync.dma_start(out=d_fw[:, :], in_=FW_b[0:1, :])
            nc.sync.dma_start(out=d_nrw[:, :], in_=nrw)
            nc.sync.dma_start(out=d_roT[:, :], in_=roT)
        # ============ Stage F: output projection ============
        wout_t = []
        for k in range(KC):
            wt_k = cons.tile([128, 1024], BF16, name=f"wout{k}")
            nc.sync.dma_start(out=wt_k, in_=wout[:, k * 1024:(k + 1) * 1024])
            wout_t.append(wt_k)
        for o in range(8):
            ps_o = ppt.tile([128, BL], dt, tag="tr")
            for k in range(KC):
                rhs = hT[:, k, :] if k < KH else \
                    roT[:, (k - KH) * 32:(k - KH + 1) * 32]
                nc.tensor.matmul(ps_o, lhsT=wout_t[k][:, o * 128:(o + 1) * 128],
                                 rhs=rhs, start=(k == 0), stop=(k == KC - 1))
            oc = wk.tile([128, BL], dt, tag="oc", bufs=2)
            nc.scalar.activation(out=oc, in_=ps_o, func=AF.Identity,
                                 bias=boc_sb[:, o:o + 1])
            nc.sync.dma_start(out=outT[o], in_=oc)

    nc.finalize()
    return nc


def _prep_inputs(x, memory, usage, link, W_ih, W_hh, b_ih, b_hh, W_if, b_if,
                 W_out, b_out):
    f = np.float32
    x = np.asarray(x, f); memory = np.asarray(memory, f)
    usage = np.asarray(usage, f); link = np.asarray(link, f)
    W_ih = np.asarray(W_ih, f); b_ih = np.asarray(b_ih, f)
    b_hh = np.asarray(b_hh, f); W_if = np.asarray(W_if, f)
    b_if = np.asarray(b_if, f); W_out = np.asarray(W_out, f)
    b_out = np.asarray(b_out, f)

    sel = np.r_[0:1024, 2048:4096]
    W3T = W_ih[sel].T                             # (1536, 3072)
    w3 = np.ascontiguousarray(
        W3T.reshape(KC, 128, 24, 128).transpose(1, 2, 0, 3)
        .reshape(128, KC * 3072).astype(ml_dtypes.bfloat16))
    b3 = (b_ih + b_hh)[sel]
    b3c = np.ascontiguousarray(b3.reshape(24, 128).T)
    wif = np.ascontiguousarray(
        W_if.T.reshape(KH, 128, IF).transpose(1, 0, 2)
        .reshape(128, KH * IF).astype(ml_dtypes.bfloat16))
    wout = np.ascontiguousarray(
        W_out.T.reshape(KC, 128, 1024).transpose(1, 0, 2)
        .reshape(128, KC * 1024).astype(ml_dtypes.bfloat16))
    boc = np.ascontiguousarray(b_out.reshape(8, 128).T)
    memA = np.ascontiguousarray(
        memory.reshape(NT, 128, 128).transpose(1, 0, 2).reshape(128, N))
    bifr = b_if.reshape(1, IF)
    usg = usage.reshape(1, N)

    shared = dict(w3=w3, b3c=b3c, wif=wif, bifr=bifr, wout=wout, boc=boc,
                  memA=memA, usg=usg)
    in_maps = []
    for c in range(M):
        xs = x[c * BL:(c + 1) * BL]               # (32, 1024)
        cinx = np.ascontiguousarray(
            xs.T.reshape(KH, 128, BL).transpose(1, 0, 2)
            .reshape(128, KH * BL).astype(ml_dtypes.bfloat16))
        ls = link[c * 256:(c + 1) * 256]          # (256, 2048)
        lnkm = np.ascontiguousarray(
            ls.reshape(2, 128, N).transpose(1, 0, 2).reshape(128, 2 * N))
        ucols = np.ascontiguousarray(
            usage.reshape(NT, 128)[2 * c:2 * c + 2].T)      # (128, 2)
        m = dict(shared)
        m["cinx"] = cinx
        m["lnk"] = lnkm
        m["ucols"] = ucols
        in_maps.append(m)
    return in_maps


def kernel(**inputs):
    nc = build_nc()
    in_maps = _prep_inputs(**inputs)
    res = run_bass_kernel_spmd(nc, in_maps, list(range(M))).results
    outs = []
    for c in range(M):
        oT = res[c]["outT"]                       # (8, 128, 32)
        outs.append(np.transpose(oT, (2, 0, 1)).reshape(BL, 1024))
    return np.concatenate(outs, 0).astype(np.float32)


# revision 22
# speedup vs baseline: 1.9885x; 1.0467x over previous
"""DNC single-step forward on 8 Trainium2 NeuronCores (Bass/Tile).

Data-parallel over batch (B=256 -> 32/core); memory/link/usage/params
replicated. Cross-core collectives:
  - one AllGather: [allocation-weight shard | link row-sum shard]
  - one AllReduce: [erase/add matrix partials | link col-sum partial |
    masked-lu sum partial]

Math restructuring (validated vs reference at ~1e-6 in numpy):
  - h0=c0=0  =>  W_hh and the forget gate are dead.
  - prev_rw uniform 1/N  =>  read_flat = tiled col-mean(memory); the
    (B,R,N)@(N,N) backward/forward einsums collapse to (1/N)*col/row-sums
    of link_new.
  - lu = ww^T ww / B is symmetric; its diag-masked row/col sums reduce to
    (w^T s - sum_b w^2)/B with s = row-sums of w; N x N lu never formed.
  - allocation weights sort-free: alloc[i] = u[i]*exp(sum_{u_k<u_i} ln(1-u_k)).
"""
import sys

sys.path.insert(0, '/opt/trn_rl_repo')

import numpy as np
import ml_dtypes
import concourse.bass as bass
import concourse.bacc as bacc
import concourse.tile as tile
from concourse import mybir
from concourse.bass_utils import run_bass_kernel_spmd
from concourse.masks import make_identity

AF = mybir.ActivationFunctionType
ALU = mybir.AluOpType
F32 = mybir.dt.float32
BF16 = mybir.dt.bfloat16

B, N, D, R, H, I = 256, 2048, 128, 4, 1024, 1024
CI = I + R * D          # 1536
IF = 787
M = 8                   # cores
BL = B // M             # 32 batch rows per core
NT = N // 128           # 16 n-tiles
KC = CI // 128          # 12 k-tiles of cin
KH = H // 128           # 8 k-tiles of h

ARA = NT * 128 * 256    # AR region A: [ep|ap] per n-tile
ARTOT = ARA + 2 * N     # + region B: [link colsum | lu stat]
DEBUG = False

# (1,N) vectors live at quarter partition offsets of two (128,N) tiles
# r0: usg@0, L@32, alloc@64, rowsum@96 ; r1: csum@0, lus@32, BW@64, FW@96


def build_nc():
    nc = bacc.Bacc("TRN2", target_bir_lowering=False, num_devices=M)
    dt = F32
    # ---- inputs (host-prepared layouts; per-partition contiguous DMA) ----
    cinx = nc.declare_dram_parameter("cinx", [128, KH * BL], BF16, isOutput=False)
    w3 = nc.declare_dram_parameter("w3", [128, KC * 3072], BF16, isOutput=False)
    b3c = nc.declare_dram_parameter("b3c", [128, 24], dt, isOutput=False)
    wif = nc.declare_dram_parameter("wif", [128, KH * IF], BF16, isOutput=False)
    bifr = nc.declare_dram_parameter("bifr", [1, IF], dt, isOutput=False)
    wout = nc.declare_dram_parameter("wout", [128, KC * 1024], BF16, isOutput=False)
    boc = nc.declare_dram_parameter("boc", [128, 8], dt, isOutput=False)
    memA = nc.declare_dram_parameter("memA", [128, N], dt, isOutput=False)
    lnk = nc.declare_dram_parameter("lnk", [128, 2 * N], dt, isOutput=False)
    usg = nc.declare_dram_parameter("usg", [1, N], dt, isOutput=False)
    ucols = nc.declare_dram_parameter("ucols", [128, 2], dt, isOutput=False)

    outT = nc.declare_dram_parameter("outT", [8, 128, BL], dt, isOutput=True)
    if DEBUG:
        d_hT = nc.declare_dram_parameter("d_hT", [128, KH * BL], dt, isOutput=True)
        d_itf = nc.declare_dram_parameter("d_itf", [32, IF], dt, isOutput=True)
        d_r0 = nc.declare_dram_parameter("d_r0", [128, N], dt, isOutput=True)
        d_mnew = nc.declare_dram_parameter("d_mnew", [128, N], dt, isOutput=True)
        d_wt = nc.declare_dram_parameter("d_wt", [32, N], dt, isOutput=True)
        d_cols = nc.declare_dram_parameter("d_cols", [128, 4], dt, isOutput=True)
        d_bw = nc.declare_dram_parameter("d_bw", [1, N], dt, isOutput=True)
        d_fw = nc.declare_dram_parameter("d_fw", [1, N], dt, isOutput=True)
        d_nrw = nc.declare_dram_parameter("d_nrw", [128, N], dt, isOutput=True)
        d_roT = nc.declare_dram_parameter("d_roT", [128, 128], dt, isOutput=True)
        d_cin = nc.declare_dram_parameter("d_cin", [128, KH * BL], dt, isOutput=True)
        d_cmean = nc.declare_dram_parameter("d_cmean", [128, BL], dt, isOutput=True)


    from contextlib import ExitStack
    with tile.TileContext(nc) as tc, ExitStack() as es:
        cons = es.enter_context(tc.tile_pool(name="cons", bufs=1))
        wk = es.enter_context(tc.tile_pool(name="wk", bufs=1))
        wstr = es.enter_context(tc.tile_pool(name="wstr", bufs=2))
        lstr = es.enter_context(tc.tile_pool(name="lstr", bufs=1))
        dpool = es.enter_context(tc.tile_pool(name="dram", bufs=1, space="DRAM"))
        pp1 = es.enter_context(tc.tile_pool(name="pp1", bufs=1, space="PSUM"))
        ppb = es.enter_context(tc.tile_pool(name="ppb", bufs=1, space="PSUM"))
        ppt = es.enter_context(tc.tile_pool(name="ppt", bufs=2, space="PSUM"))

        ag_in = dpool.tile([512], dt)
        ag_out = dpool.tile([M, 512], dt, addr_space="Shared")
        ar_in = dpool.tile([ARTOT], BF16)
        ar_out = dpool.tile([ARTOT], BF16, addr_space="Shared")
        arA_in = ar_in[0:ARA].rearrange("(t p f) -> t p f", p=128, f=256)
        arA_out = ar_out[0:ARA].rearrange("(t p f) -> t p f", p=128, f=256)

        ident = cons.tile([128, 128], dt)
        make_identity(nc, ident)
        ones_col = cons.tile([128, 1], dt)
        nc.vector.memset(ones_col, 1.0)
        ones32 = cons.tile([128, BL], dt)
        nc.vector.memset(ones32, 1.0)

        r0 = cons.tile([128, N], dt)
        r1 = cons.tile([128, N], dt)
        nc.sync.dma_start(out=r0[0:1, :], in_=usg[:, :])
        bif_sb = cons.tile([1, IF], dt)
        nc.sync.dma_start(out=bif_sb, in_=bifr[:, :])
        b3_sb = cons.tile([128, 24], dt)
        nc.sync.dma_start(out=b3_sb, in_=b3c[:, :])
        boc_sb = cons.tile([128, 8], dt)
        nc.sync.dma_start(out=boc_sb, in_=boc[:, :])
        mem_sb = cons.tile([128, N], dt)
        nc.sync.dma_start(out=mem_sb, in_=memA[:, :])
        uc_sb = cons.tile([128, 2], dt)
        nc.sync.dma_start(out=uc_sb, in_=ucols[:, :])

        usg_row = r0[0:1, :]
        tmp_row = r0[0:1, :]  # reused in stage E (usage dead by then)

        # ============ Stage A: alloc shard + link stats -> AllGather ========
        ones_row = cons.tile([1, 128], dt)
        nc.vector.memset(ones_row, 1.0)
        usg_b = wk.tile([128, N], dt, tag="usgb")
        bc_ps = ppb.tile([128, N], dt, tag="big")
        for ch in range(4):
            nc.tensor.matmul(bc_ps[:, ch * 512:(ch + 1) * 512], lhsT=ones_row,
                             rhs=usg_row[:, ch * 512:(ch + 1) * 512],
                             start=True, stop=True)
        nc.vector.tensor_copy(out=usg_b, in_=bc_ps)
        L_b = wk.tile([128, N], dt, tag="lb")
        nc.scalar.activation(out=L_b, in_=usg_b, func=AF.Ln, bias=1.0,
                             scale=-1.0)
        for i in range(2):
            u_col = uc_sb[:, i:i + 1]
            step = wk.tile([128, N], dt, tag="step", bufs=1)
            nc.vector.tensor_scalar(out=step, in0=usg_b,
                                    scalar1=u_col, scalar2=None, op0=ALU.is_lt)
            nc.vector.tensor_tensor(out=step, in0=step, in1=L_b, op=ALU.mult)
            a_col = wk.tile([128, 1], dt, tag="acol", bufs=2)
            nc.vector.tensor_reduce(out=a_col, in_=step,
                                    axis=mybir.AxisListType.X, op=ALU.add)
            nc.scalar.activation(out=a_col, in_=a_col, func=AF.Exp)
            nc.vector.tensor_mul(out=a_col, in0=a_col, in1=u_col)
            nc.sync.dma_start(out=ag_in[i * 128:(i + 1) * 128], in_=a_col)

        cs_ps = ppb.tile([1, N], dt, tag="big")
        for i in range(2):
            lt = lstr.tile([128, N], dt, tag="lnk")
            nc.sync.dma_start(out=lt, in_=lnk[:, i * N:(i + 1) * N])
            rs = wk.tile([128, 1], dt, tag="rs", bufs=2)
            nc.vector.tensor_reduce(out=rs, in_=lt, axis=mybir.AxisListType.X,
                                    op=ALU.add)
            nc.sync.dma_start(out=ag_in[256 + i * 128:256 + (i + 1) * 128],
                              in_=rs)
            for ch in range(4):
                nc.tensor.matmul(cs_ps[0:1, ch * 512:(ch + 1) * 512],
                                 lhsT=ones_col,
                                 rhs=lt[:, ch * 512:(ch + 1) * 512],
                                 start=(i == 0), stop=(i == 1))
        cs_row = wk.tile([1, N], BF16, tag="csrow")
        nc.scalar.copy(out=cs_row, in_=cs_ps)
        nc.sync.dma_start(out=ar_in[ARA:ARA + N], in_=cs_row)

        nc.gpsimd.collective_compute(
            "AllGather", ALU.bypass, replica_groups=[list(range(M))],
            ins=[ag_in[:]], outs=[ag_out.flatten()])
        alloc_r = cons.tile([1, N], dt)
        nc.sync.dma_start(out=alloc_r, in_=ag_out[:, 0:256])

        # ============ Stage B: LSTM ============
        cin = wk.tile([128, KH, BL], BF16)
        nc.sync.dma_start(out=cin,
                          in_=cinx[:, :].rearrange("p (k b) -> p k b", b=BL))
        mean_ps = ppt.tile([1, 128], dt, tag="tr")
        for t in range(NT):
            nc.tensor.matmul(mean_ps, lhsT=ones_col,
                             rhs=mem_sb[:, t * 128:(t + 1) * 128],
                             start=(t == 0), stop=(t == NT - 1))
        mean_row = wk.tile([1, 128], dt, tag="meanr")
        nc.scalar.activation(out=mean_row, in_=mean_ps, func=AF.Copy,
                             scale=1.0 / N)
        mc_ps = ppt.tile([128, 1], dt, tag="tr")
        nc.tensor.transpose(mc_ps, mean_row, ident[0:1, 0:1])
        mean_col = wk.tile([128, 1], dt, tag="meanc")
        nc.vector.tensor_copy(out=mean_col, in_=mc_ps)
        cmean = wk.tile([128, BL], BF16)
        nc.scalar.activation(out=cmean, in_=ones32, func=AF.Copy,
                             scale=mean_col)

        # j-outer / k-inner: one psum bank per accumulation group (a
        # start=True matmul claims a whole 2KB zero-region, so slices of one
        # bank cannot host interleaved groups).
        gsb = wk.tile([128, 24, BL], dt)   # activated gates, j-major
        for j in range(24):
            w3j = wstr.tile([128, KC * 128], BF16, tag="w3j", bufs=4)
            nc.sync.dma_start(out=w3j,
                              in_=w3[:, j * KC * 128:(j + 1) * KC * 128])
            ps_j = ppt.tile([128, BL], dt, tag="tr")
            for k in range(KC):
                rhs_k = cin[:, k, :] if k < KH else cmean
                nc.tensor.matmul(ps_j, lhsT=w3j[:, k * 128:(k + 1) * 128],
                                 rhs=rhs_k,
                                 start=(k == 0), stop=(k == KC - 1))
            fn = AF.Tanh if 8 <= j < 16 else AF.Sigmoid
            nc.scalar.activation(out=gsb[:, j, :], in_=ps_j, func=fn,
                                 bias=b3_sb[:, j:j + 1])
        hT = wk.tile([128, KH, BL], BF16)
        for t in range(KH):
            cc = wk.tile([128, BL], dt, tag="g1", bufs=2)
            nc.vector.tensor_mul(out=cc, in0=gsb[:, t, :], in1=gsb[:, 8 + t, :])
            nc.scalar.activation(out=cc, in_=cc, func=AF.Tanh)
            nc.vector.tensor_mul(out=hT[:, t, :], in0=cc, in1=gsb[:, 16 + t, :])

        # ============ Stage C: interface vector ============
        ps_itf = ppb.tile([32, IF], dt, tag="big")
        for k in range(KH):
            wfk = wstr.tile([128, IF], BF16, tag="wifk")
            nc.sync.dma_start(out=wfk, in_=wif[:, k * IF:(k + 1) * IF])
            nc.tensor.matmul(ps_itf[:, 0:512], lhsT=hT[:, k, :],
                             rhs=wfk[:, 0:512], start=(k == 0),
                             stop=(k == KH - 1))
            nc.tensor.matmul(ps_itf[:, 512:IF], lhsT=hT[:, k, :],
                             rhs=wfk[:, 512:IF], start=(k == 0),
                             stop=(k == KH - 1))
        bif_b = wk.tile([32, IF], dt)
        nc.sync.dma_start(out=bif_b, in_=bifr[0:1, :].partition_broadcast(32))
        itf = wk.tile([32, IF], dt)
        nc.vector.tensor_tensor(out=itf, in0=ps_itf, in1=bif_b, op=ALU.add)

        wv = itf[:, 0:128]
        ersig = wk.tile([32, 128], dt)
        nc.scalar.activation(out=ersig, in_=itf[:, 128:256], func=AF.Sigmoid)
        wgag = wk.tile([32, 2], dt)
        nc.scalar.activation(out=wgag, in_=itf[:, 256:258], func=AF.Sigmoid)
        wg = wgag[:, 0:1]
        agt = wgag[:, 1:2]
        expm = wk.tile([32, 12], dt)
        nc.scalar.activation(out=expm, in_=itf[:, 259:271], func=AF.Exp)
        msum = wk.tile([32, 4], dt)
        nc.vector.tensor_reduce(out=msum,
                                in_=expm.rearrange("p (r k) -> p r k", k=3),
                                axis=mybir.AxisListType.X, op=ALU.add)
        minv = wk.tile([32, 4], dt)
        nc.vector.reciprocal(out=minv, in_=msum)
        sc16 = wk.tile([32, 16], dt)   # [rstr | m0 | m1 | m2]
        nc.scalar.activation(out=sc16[:, 0:4], in_=itf[:, 271:275],
                             func=AF.Exp)
        nc.scalar.activation(out=sc16[:, 0:4], in_=sc16[:, 0:4],
                             func=AF.Ln, bias=1.0)
        em3 = expm.rearrange("p (r k) -> p r k", k=3)
        for kk in range(3):
            nc.vector.tensor_mul(out=sc16[:, 4 + 4 * kk:8 + 4 * kk],
                                 in0=em3[:, :, kk], in1=minv)
        ps_t16 = ppt.tile([16, 32], dt, tag="tr")
        nc.tensor.transpose(ps_t16, sc16, ident[0:32, 0:32])
        t16 = wk.tile([16, 32], dt)
        nc.vector.tensor_copy(out=t16, in_=ps_t16)
        cols4 = wk.tile([128, 4], dt)  # [str | m0 | m1 | m2] as rb-columns
        for q in range(4):
            nc.sync.dma_start(out=cols4[:, q:q + 1],
                              in_=t16[4 * q:4 * q + 4, :])
        str_col = cols4[:, 0:1]
        m0_col = cols4[:, 1:2]
        m1_col = cols4[:, 2:3]
        m2_col = cols4[:, 3:4]

        ev = wk.tile([32, 128], dt)
        nc.vector.tensor_scalar(out=ev, in0=ersig, scalar1=wg, scalar2=None,
                                op0=ALU.mult)
        av = wk.tile([32, 128], dt)
        nc.vector.tensor_scalar(out=av, in0=wv, scalar1=wg, scalar2=None,
                                op0=ALU.mult)

        sq = wk.tile([32, 128], dt, tag="sq")
        nrm = wk.tile([32, 1], dt, tag="nrm")
        nc.scalar.activation(out=sq, in_=wv, func=AF.Square, accum_out=nrm)
        nc.scalar.activation(out=nrm, in_=nrm, func=AF.Sqrt)
        nc.vector.tensor_scalar(out=nrm, in0=nrm, scalar1=1e-12, scalar2=None,
                                op0=ALU.max)
        nc.vector.reciprocal(out=nrm, in_=nrm)
        nwv = wk.tile([32, 128], dt)
        nc.vector.tensor_scalar(out=nwv, in0=wv, scalar1=nrm, scalar2=None,
                                op0=ALU.mult)
        ps_nwvT = ppt.tile([128, 32], dt, tag="tr")
        nc.tensor.transpose(ps_nwvT, nwv, ident[0:32, 0:32])
        nwvT = wk.tile([128, 32], dt)
        nc.vector.tensor_copy(out=nwvT, in_=ps_nwvT)

        # ============ Stage D: write addressing + partials -> AllReduce =====
        memnT = wk.tile([128, N], dt, tag="memnT", bufs=1)
        mem3 = mem_sb.rearrange("p (t d) -> p t d", d=128)
        sqm3 = wk.tile([128, NT, 128], dt, tag="step", bufs=1)
        nc.vector.tensor_mul(out=sqm3, in0=mem3, in1=mem3)
        mn16 = wk.tile([128, NT], dt)
        nc.vector.tensor_reduce(out=mn16, in_=sqm3, axis=mybir.AxisListType.X,
                                op=ALU.add)
        nc.scalar.activation(out=mn16, in_=mn16, func=AF.Sqrt)
        nc.vector.tensor_scalar(out=mn16, in0=mn16, scalar1=1e-12,
                                scalar2=None, op0=ALU.max)
        nc.vector.reciprocal(out=mn16, in_=mn16)
        nc.vector.tensor_tensor(out=sqm3, in0=mem3,
                                in1=mn16.unsqueeze(2).broadcast_to([128, NT, 128]),
                                op=ALU.mult)
        for t in range(NT):
            ps_tr = ppt.tile([128, 128], dt, tag="tr")
            nc.tensor.transpose(ps_tr, sqm3[:, t, :], ident)
            nc.vector.tensor_copy(out=memnT[:, t * 128:(t + 1) * 128], in_=ps_tr)

        ps_cw = ppb.tile([32, N], dt, tag="big")
        for ch in range(4):
            nc.tensor.matmul(ps_cw[:, ch * 512:(ch + 1) * 512], lhsT=nwvT,
                             rhs=memnT[:, ch * 512:(ch + 1) * 512],
                             start=True, stop=True)
        cwexp = wk.tile([32, N], dt)
        den = wk.tile([32, 1], dt)
        nc.scalar.activation(out=cwexp, in_=ps_cw, func=AF.Exp, accum_out=den)
        nc.vector.reciprocal(out=den, in_=den)
        a_sc = wk.tile([32, 1], dt)
        nc.vector.tensor_mul(out=a_sc, in0=wg, in1=den)
        nc.vector.tensor_scalar(out=a_sc, in0=a_sc, scalar1=0.5, scalar2=None,
                                op0=ALU.mult)
        b_sc = wk.tile([32, 1], dt)
        nc.vector.tensor_mul(out=b_sc, in0=wg, in1=agt)
        nc.vector.tensor_scalar(out=b_sc, in0=b_sc, scalar1=0.5, scalar2=None,
                                op0=ALU.mult)
        ps_bt = ppt.tile([1, 32], dt, tag="trq", bufs=1)
        nc.tensor.transpose(ps_bt, b_sc, ident[0:32, 0:32])
        b_scT = wk.tile([1, 32], dt)
        nc.vector.tensor_copy(out=b_scT, in_=ps_bt)
        ps_w = ppb.tile([32, N], dt, tag="big")
        for ch in range(4):
            nc.tensor.matmul(ps_w[:, ch * 512:(ch + 1) * 512], lhsT=b_scT,
                             rhs=alloc_r[:, ch * 512:(ch + 1) * 512],
                             start=True, stop=True)
        wt = cwexp
        for ch in range(4):
            nc.vector.scalar_tensor_tensor(
                out=wt[:, ch * 512:(ch + 1) * 512],
                in0=cwexp[:, ch * 512:(ch + 1) * 512], scalar=a_sc,
                in1=ps_w[:, ch * 512:(ch + 1) * 512], op0=ALU.mult, op1=ALU.add)
        wsq = wk.tile([32, N], dt)
        nc.vector.tensor_mul(out=wsq, in0=wt, in1=wt)
        s_col = wk.tile([32, 1], dt)
        nc.vector.tensor_reduce(out=s_col, in_=wt, axis=mybir.AxisListType.X,
                                op=ALU.add)
        rhs_eva = wk.tile([32, 257], dt)
        nc.vector.tensor_copy(out=rhs_eva[:, 0:128], in_=ev)
        nc.vector.tensor_copy(out=rhs_eva[:, 128:256], in_=av)
        nc.vector.tensor_copy(out=rhs_eva[:, 256:257], in_=s_col)
        stag = wk.tile([128, NT, 256], BF16)
        lustag = wk.tile([128, NT], BF16)
        for t in range(NT):
            ps_p = ppt.tile([128, 257], dt, tag="tr")
            nc.tensor.matmul(ps_p, lhsT=wt[:, t * 128:(t + 1) * 128],
                             rhs=rhs_eva, start=True, stop=True)
            ps_q = ppt.tile([128, 1], dt, tag="trq", bufs=1)
            nc.tensor.matmul(ps_q, lhsT=wsq[:, t * 128:(t + 1) * 128],
                             rhs=ones_col[0:32, :], start=True, stop=True)
            nc.vector.tensor_copy(out=stag[:, t, :], in_=ps_p[:, 0:256])
            qsb = wk.tile([128, 1], dt, tag="qsb", bufs=2)
            nc.vector.tensor_copy(out=qsb, in_=ps_q)
            nc.vector.tensor_sub(out=lustag[:, t:t + 1], in0=ps_p[:, 256:257],
                                 in1=qsb)
        nc.sync.dma_start(
            out=ar_in[0:ARA].rearrange("(t p f) -> p t f", p=128, f=256),
            in_=stag)
        nc.sync.dma_start(
            out=ar_in[ARA + N:ARA + 2 * N].rearrange("(t p) -> p t", p=128),
            in_=lustag)

        nc.gpsimd.collective_compute(
            "AllReduce", ALU.add, replica_groups=[list(range(M))],
            ins=[ar_in[:]], outs=[ar_out[:]])

        # ============ Stage E: memory update + read weights ============
        # [csum ; rowsum] rows -> BW/FW rows (2,N); lus row doubled to (2,N)
        cr2 = wk.tile([2, N], dt)
        csb = wk.tile([1, N], BF16, tag="csrow")
        nc.sync.dma_start(out=csb, in_=ar_out[ARA:ARA + N])
        nc.vector.tensor_copy(out=cr2[0:1, :], in_=csb)
        nc.sync.dma_start(out=cr2[1:2, :], in_=ag_out[:, 256:512])
        lu2 = wk.tile([2, N], BF16)
        nc.sync.dma_start(out=lu2[0:1, :], in_=ar_out[ARA + N:ARA + 2 * N])
        nc.sync.dma_start(out=lu2[1:2, :], in_=ar_out[ARA + N:ARA + 2 * N])
        bwfw = wk.tile([2, N], dt)
        nc.vector.tensor_scalar(out=bwfw, in0=cr2, scalar1=0.9 / N,
                                scalar2=None, op0=ALU.mult)
        nc.vector.scalar_tensor_tensor(out=bwfw, in0=lu2,
                                       scalar=0.1 / (N * B), in1=bwfw,
                                       op0=ALU.mult, op1=ALU.add)

        ea_full = wk.tile([128, NT, 256], BF16, tag="eaf")
        nc.sync.dma_start(
            out=ea_full,
            in_=ar_out[0:ARA].rearrange("(t p f) -> p t f", p=128, f=256))
        mnew = wk.tile([128, N], dt)
        mnew3 = mnew.rearrange("p (t d) -> p t d", d=128)
        f3 = wk.tile([128, NT, 128], dt, tag="step", bufs=1)
        nc.vector.tensor_scalar(out=f3, in0=ea_full[:, :, 0:128],
                                scalar1=-1.0 / B, scalar2=1.0, op0=ALU.mult,
                                op1=ALU.add)
        nc.vector.tensor_mul(out=f3, in0=f3,
                             in1=mem_sb.rearrange("p (t d) -> p t d", d=128))
        nc.vector.scalar_tensor_tensor(out=mnew3, in0=ea_full[:, :, 128:256],
                                       scalar=1.0 / B, in1=f3, op0=ALU.mult,
                                       op1=ALU.add)
        # batched row norms of mnew
        sqf = f3
        nc.vector.tensor_mul(out=sqf, in0=mnew3, in1=mnew3)
        nrm16 = wk.tile([128, NT], dt)
        nc.vector.tensor_reduce(out=nrm16, in_=sqf, axis=mybir.AxisListType.X,
                                op=ALU.add)
        nc.scalar.activation(out=nrm16, in_=nrm16, func=AF.Sqrt)
        nc.vector.tensor_scalar(out=nrm16, in0=nrm16, scalar1=1e-12,
                                scalar2=None, op0=ALU.max)
        nc.vector.reciprocal(out=nrm16, in_=nrm16)
        nmn = wk.tile([128, NT, 128], dt, tag="lb", bufs=1)
        nc.vector.tensor_tensor(out=nmn, in0=mnew3,
                                in1=nrm16.unsqueeze(2).broadcast_to([128, NT, 128]),
                                op=ALU.mult)
        mnewT = wk.tile([128, N], dt, tag="memnT", bufs=1)
        for t in range(NT):
            ps_tr = ppt.tile([128, 128], dt, tag="tr")
            nc.tensor.transpose(ps_tr, nmn[:, t, :], ident)
            nc.vector.tensor_copy(out=mnewT[:, t * 128:(t + 1) * 128], in_=ps_tr)

        nkT = wk.tile([128, 128], dt)
        rk3 = itf[:, 275:787].rearrange("p (r d) -> p r d", d=128)
        sqk3 = wk.tile([32, R, 128], dt)
        nc.vector.tensor_mul(out=sqk3, in0=rk3, in1=rk3)
        nrk4 = wk.tile([32, R], dt)
        nc.vector.tensor_reduce(out=nrk4, in_=sqk3, axis=mybir.AxisListType.X,
                                op=ALU.add)
        nc.scalar.activation(out=nrk4, in_=nrk4, func=AF.Sqrt)
        nc.vector.tensor_scalar(out=nrk4, in0=nrk4, scalar1=1e-12,
                                scalar2=None, op0=ALU.max)
        nc.vector.reciprocal(out=nrk4, in_=nrk4)
        nc.vector.tensor_tensor(out=sqk3, in0=rk3,
                                in1=nrk4.unsqueeze(2).broadcast_to([32, R, 128]),
                                op=ALU.mult)
        for r in range(R):
            ps_k = ppt.tile([128, 32], dt, tag="tr")
            nc.tensor.transpose(ps_k, sqk3[:, r, :], ident[0:32, 0:32])
            nc.vector.tensor_copy(out=nkT[:, r * 32:(r + 1) * 32], in_=ps_k)

        ps_sim = ppb.tile([128, N], dt, tag="big")
        for ch in range(4):
            nc.tensor.matmul(ps_sim[:, ch * 512:(ch + 1) * 512], lhsT=nkT,
                             rhs=mnewT[:, ch * 512:(ch + 1) * 512],
                             start=True, stop=True)
        esim = wk.tile([128, N], dt)
        dsum = wk.tile([128, 1], dt)
        nc.scalar.activation(out=esim, in_=ps_sim, func=AF.Exp, scale=str_col,
                             accum_out=dsum)
        nc.vector.reciprocal(out=dsum, in_=dsum)
        c0 = wk.tile([128, 1], dt)
        nc.vector.tensor_mul(out=c0, in0=m0_col, in1=dsum)
        ps_mt = ppt.tile([2, 128], dt, tag="trq", bufs=1)
        nc.tensor.transpose(ps_mt, cols4[:, 2:4], ident)
        m12T = wk.tile([2, 128], dt)
        nc.vector.tensor_copy(out=m12T, in_=ps_mt)
        ps_term = ppb.tile([128, N], dt, tag="big")
        for ch in range(4):
            nc.tensor.matmul(ps_term[:, ch * 512:(ch + 1) * 512], lhsT=m12T,
                             rhs=bwfw[:, ch * 512:(ch + 1) * 512],
                             start=True, stop=True)
        nrw = esim
        for ch in range(4):
            nc.vector.scalar_tensor_tensor(
                out=nrw[:, ch * 512:(ch + 1) * 512],
                in0=esim[:, ch * 512:(ch + 1) * 512], scalar=c0,
                in1=ps_term[:, ch * 512:(ch + 1) * 512], op0=ALU.mult,
                op1=ALU.add)

        ps_ro = pp1.tile([128, 128], dt, tag="psA")
        roT = wk.tile([128, 128], BF16)
        for t in range(NT):
            ps_tr = ppt.tile([128, 128], dt, tag="tr")
            nc.tensor.transpose(ps_tr, nrw[:, t * 128:(t + 1) * 128], ident)
            nrwT = wk.tile([128, 128], dt, tag="nrwT", bufs=2)
            nc.vector.tensor_copy(out=nrwT, in_=ps_tr)
            nc.tensor.matmul(ps_ro, lhsT=mnew[:, t * 128:(t + 1) * 128],
                             rhs=nrwT, start=(t == 0), stop=(t == NT - 1))
        nc.vector.tensor_copy(out=roT, in_=ps_ro)

        if DEBUG:
            nc.sync.dma_start(out=d_cin[:, :], in_=cin.rearrange("p k b -> p (k b)"))
            nc.sync.dma_start(out=d_cmean[:, :], in_=cmean)
            nc.sync.dma_start(out=d_hT[:, :], in_=hT.rearrange("p k b -> p (k b)"))
            nc.sync.dma_start(out=d_itf[:, :], in_=itf)
            nc.sync.dma_start(out=d_r0[:, :], in_=r0)
            nc.sync.dma_start(out=d_mnew[:, :], in_=mnew)
            nc.sync.dma_start(out=d_wt[:, :], in_=wt)
            nc.sync.dma_start(out=d_cols[:, :], in_=cols4)
            nc.sync.dma_start(out=d_bw[:, :], in_=BW_b[0:1, :])
            nc.sync.dma_start(out=d_fw[:, :], in_=FW_b[0:1, :])
            nc.sync.dma_start(out=d_nrw[:, :], in_=nrw)
            nc.sync.dma_start(out=d_roT[:, :], in_=roT)
        # ============ Stage F: output projection ============
        wout_t = []
        for k in range(KC):
            wt_k = cons.tile([128, 1024], BF16, name=f"wout{k}")
            nc.sync.dma_start(out=wt_k, in_=wout[:, k * 1024:(k + 1) * 1024])
            wout_t.append(wt_k)
        for o in range(8):
            ps_o = ppt.tile([128, BL], dt, tag="tr")
            for k in range(KC):
                rhs = hT[:, k, :] if k < KH else \
                    roT[:, (k - KH) * 32:(k - KH + 1) * 32]
                nc.tensor.matmul(ps_o, lhsT=wout_t[k][:, o * 128:(o + 1) * 128],
                                 rhs=rhs, start=(k == 0), stop=(k == KC - 1))
            oc = wk.tile([128, BL], dt, tag="oc", bufs=2)
            nc.scalar.activation(out=oc, in_=ps_o, func=AF.Identity,
                                 bias=boc_sb[:, o:o + 1])
            nc.sync.dma_start(out=outT[o], in_=oc)

    nc.finalize()
    return nc


def _prep_inputs(x, memory, usage, link, W_ih, W_hh, b_ih, b_hh, W_if, b_if,
                 W_out, b_out):
    f = np.float32
    x = np.asarray(x, f); memory = np.asarray(memory, f)
    usage = np.asarray(usage, f); link = np.asarray(link, f)
    W_ih = np.asarray(W_ih, f); b_ih = np.asarray(b_ih, f)
    b_hh = np.asarray(b_hh, f); W_if = np.asarray(W_if, f)
    b_if = np.asarray(b_if, f); W_out = np.asarray(W_out, f)
    b_out = np.asarray(b_out, f)

    sel = np.r_[0:1024, 2048:4096]
    W3T = W_ih[sel].T                             # (1536, 3072)
    w3 = np.ascontiguousarray(
        W3T.reshape(KC, 128, 24, 128).transpose(1, 2, 0, 3)
        .reshape(128, KC * 3072).astype(ml_dtypes.bfloat16))
    b3 = (b_ih + b_hh)[sel]
    b3c = np.ascontiguousarray(b3.reshape(24, 128).T)
    wif = np.ascontiguousarray(
        W_if.T.reshape(KH, 128, IF).transpose(1, 0, 2)
        .reshape(128, KH * IF).astype(ml_dtypes.bfloat16))
    wout = np.ascontiguousarray(
        W_out.T.reshape(KC, 128, 1024).transpose(1, 0, 2)
        .reshape(128, KC * 1024).astype(ml_dtypes.bfloat16))
    boc = np.ascontiguousarray(b_out.reshape(8, 128).T)
    memA = np.ascontiguousarray(
        memory.reshape(NT, 128, 128).transpose(1, 0, 2).reshape(128, N))
    bifr = b_if.reshape(1, IF)
    usg = usage.reshape(1, N)

    shared = dict(w3=w3, b3c=b3c, wif=wif, bifr=bifr, wout=wout, boc=boc,
                  memA=memA, usg=usg)
    in_maps = []
    for c in range(M):
        xs = x[c * BL:(c + 1) * BL]               # (32, 1024)
        cinx = np.ascontiguousarray(
            xs.T.reshape(KH, 128, BL).transpose(1, 0, 2)
            .reshape(128, KH * BL).astype(ml_dtypes.bfloat16))
        ls = link[c * 256:(c + 1) * 256]          # (256, 2048)
        lnkm = np.ascontiguousarray(
            ls.reshape(2, 128, N).transpose(1, 0, 2).reshape(128, 2 * N))
        ucols = np.ascontiguousarray(
            usage.reshape(NT, 128)[2 * c:2 * c + 2].T)      # (128, 2)
        m = dict(shared)
        m["cinx"] = cinx
        m["lnk"] = lnkm
        m["ucols"] = ucols
        in_maps.append(m)
    return in_maps


def kernel(**inputs):
    nc = build_nc()
    in_maps = _prep_inputs(**inputs)
    res = run_bass_kernel_spmd(nc, in_maps, list(range(M))).results
    outs = []
    for c in range(M):
        oT = res[c]["outT"]                       # (8, 128, 32)
        outs.append(np.transpose(oT, (2, 0, 1)).reshape(BL, 1024))
    return np.concatenate(outs, 0).astype(np.float32)
